# revision 1
# baseline (speedup 1.0000x reference)
"""DeeperGCN forward on 8 Trainium2 NeuronCores (Bass/Tile).

Strategy (graph/data parallel, dst-node sharding):
- Nodes sharded 6250/core. Edges assigned to the core owning their dst,
  sorted by dst, tiled 128/tile within 128-node dst blocks.
- Per conv layer, each core computes node tables P1 = exp(t*msg),
  P2 = P1*msg (msg = relu(r)+eps) for its own nodes; tables are
  AllGathered (fp16) so every core holds the full [N,128] T=[P2|P1].
- Edge phase: indirect-DMA gather of T rows by src, one-hot matmul
  aggregation (onehot[e,dst].T @ T_rows) accumulated in PSUM per dst
  block -> num/den -> agg = num/max(den,1e-16).  This reproduces the
  softmax aggregation exactly (max-subtraction cancels in num/den).
- Node phase: root residual + MLP (Linear->LN/BN->ReLU->Linear) with
  TensorE matmuls/transposes, DVE/ACT elementwise, node-major layout.
"""

import json
import sys
import types

import numpy as np

sys.path.insert(0, "/opt/trn_rl_repo")

# ---------------------------------------------------------------------------
# Workaround: this walrus build supports only ONE semaphore wait per
# instruction; Tile attaches several. Split extras onto NoOp instructions
# at BIR-JSON serialization time.
# ---------------------------------------------------------------------------
_PATCHED = False


def _install_bir_patch():
    global _PATCHED
    if _PATCHED:
        return
    _PATCHED = True
    import concourse.bass as bass

    orig = bass.Bass.to_json_bytes

    def patched(self):
        data = json.loads(orig(self).decode())
        ctr = 0
        for fn in data.get("functions", []):
            for bb in fn.get("blocks", []):
                new_insts = []
                for inst in bb.get("instructions", []):
                    si = inst.get("sync_info")
                    waits = (si or {}).get("on_wait") or []
                    if len(waits) > 1:
                        for w in waits[:-1]:
                            ctr += 1
                            nop = {
                                "engine": inst["engine"],
                                "ins": [],
                                "outs": [],
                                "name": f"{inst['name']}-sw{ctr}",
                                "opcode": "NoOp",
                                "sync_info": {"on_update": [], "on_wait": [w]},
                            }
                            if "debug" in inst:
                                nop["debug"] = inst["debug"]
                            new_insts.append(nop)
                        si["on_wait"] = [waits[-1]]
                    new_insts.append(inst)
                bb["instructions"] = new_insts
        return json.dumps(data).encode()

    bass.Bass.to_json_bytes = patched


def _install_trace_hook():
    """Optional: register the NTFF profiling hook (for test.py timing)."""
    import antenv

    if "antenv.axon_hooks" in sys.modules:
        return
    _m = types.ModuleType("antenv.axon_hooks")
    _m._hook = None
    _m.set_axon_ntff_profile_hook = lambda h: setattr(_m, "_hook", h)
    _m.get_axon_ntff_profile_hook = lambda: _m._hook
    sys.modules["antenv.axon_hooks"] = _m
    antenv.axon_hooks = _m
    try:
        from trn_agent_boot.trn_boot import _ntff_profile_via_ctypes

        _m._hook = _ntff_profile_via_ctypes("/opt/axon/libaxon_pjrt.so")
    except Exception:
        pass


N, NC, NPC = 50000, 8, 6250
H = 64
H2 = 128
F_IN = 128
LN_EPS = 1e-5
BN_EPS = 1e-5
GEN_EPS = 1e-7

LAST_EXEC_NS = None


def _preprocess_edges(edge_index):
    """Per-core dst-sorted edges, variable node-range blocks with a fixed
    2048-edge budget (16 tiles of 128) per block. Returns shared tile
    geometry + per-core index arrays + per-core block node bounds."""
    CAP = 2048
    TPB = CAP // 128  # 16 tiles per block
    src = np.asarray(edge_index[0], dtype=np.int64)
    dst = np.asarray(edge_index[1], dtype=np.int64)
    core_edges = []
    core_qb = []
    for c in range(NC):
        m = (dst >= c * NPC) & (dst < (c + 1) * NPC)
        s_c = src[m]
        d_c = dst[m] - c * NPC
        order = np.argsort(d_c, kind="stable")
        s_c, d_c = s_c[order], d_c[order]
        deg = np.bincount(d_c, minlength=NPC)
        qb = [0]
        nodes_in = 0
        cum = 0
        for n in range(NPC):
            dn = int(deg[n])
            if nodes_in == 128 or cum + dn > CAP:
                qb.append(n)
                nodes_in = 0
                cum = 0
            nodes_in += 1
            cum += dn
        qb.append(NPC)
        core_edges.append((s_c, d_c))
        core_qb.append(qb)
    nblk = max(len(qb) - 1 for qb in core_qb)
    nblk += nblk % 2  # even block count for half-split AllGather
    # pad bounds to nblk+1 entries (trailing empty blocks)
    qbounds = np.full((NC, nblk + 1), NPC, dtype=np.int64)
    for c in range(NC):
        qb = core_qb[c]
        qbounds[c, : len(qb)] = qb
    # node -> (block, pos) per core
    blk_of = np.zeros((NC, NPC), dtype=np.int64)
    pos_of = np.zeros((NC, NPC), dtype=np.int64)
    for c in range(NC):
        for b in range(nblk):
            q0, q1 = int(qbounds[c, b]), int(qbounds[c, b + 1])
            if q1 > q0:
                blk_of[c, q0:q1] = b
                pos_of[c, q0:q1] = np.arange(q1 - q0)
    R2 = NC * 128 + 8  # rows per half-table incl zero-pad rows
    HBK = nblk // 2
    pad_row = (NC * 128) * HBK  # flat row of the zeroed pad region (h=0)
    # per-block tile count: cross-core max, so all-pad trailing tiles vanish
    core_bounds = []
    ecb = np.zeros((NC, nblk), dtype=np.int64)
    for c in range(NC):
        _, d_c = core_edges[c]
        bounds = np.searchsorted(d_c, qbounds[c])
        core_bounds.append(bounds)
        ecb[c] = bounds[1:] - bounds[:-1]
    tpb = np.maximum(1, -(-ecb.max(axis=0) // 128))  # ceil
    tile_ofs = np.concatenate([[0], np.cumsum(tpb)])
    tot = int(tile_ofs[-1])
    srcidx = np.full((NC, 128, tot), pad_row, dtype=np.int32)
    dstrel = np.full((NC, 128, tot), 255.0, dtype=np.float16)
    for c in range(NC):
        s_c, d_c = core_edges[c]
        # remap src global id -> table row index
        cs = s_c // NPC
        ls = s_c % NPC
        bk = blk_of[cs, ls]
        hh = bk // HBK
        sidx = (hh * R2 + cs * 128 + pos_of[cs, ls]) * HBK + (bk % HBK)
        bounds = core_bounds[c]
        for b in range(nblk):
            e0, e1 = int(bounds[b]), int(bounds[b + 1])
            ne = e1 - e0
            nt = int(tpb[b])
            cap_b = nt * 128
            assert ne <= cap_b
            sp = np.full(cap_b, pad_row, dtype=np.int64)
            dp = np.full(cap_b, 255, dtype=np.int64)
            sp[:ne] = sidx[e0:e1]
            dp[:ne] = d_c[e0:e1] - qbounds[c, b]
            t0 = int(tile_ofs[b])
            srcidx[c, :, t0 : t0 + nt] = sp.reshape(nt, 128).T
            dstrel[c, :, t0 : t0 + nt] = dp.reshape(nt, 128).T.astype(np.float16)
    return (
        nblk,
        tpb.astype(int).tolist(),
        tile_ofs.astype(int).tolist(),
        tot,
        srcidx,
        dstrel,
        qbounds,
        pad_row,
    )


def kernel(
    x,
    edge_index,
    enc_W,
    enc_b,
    conv_t,
    conv_W1,
    conv_b1,
    conv_lng,
    conv_lnb,
    conv_W2,
    conv_b2,
    block_lng,
    block_lnb,
    fin_t,
    fin_W1,
    fin_b1,
    fin_bng,
    fin_bnb,
    fin_W2,
    fin_b2,
    _trace=False,
):
    global LAST_EXEC_NS
    _install_bir_patch()
    if _trace:
        _install_trace_hook()

    import concourse.bass as bass
    import concourse.mybir as mybir
    import concourse.tile as tile
    from concourse.bass import IndirectOffsetOnAxis
    from concourse.bass_utils import run_bass_kernel_spmd
    from concourse.masks import make_identity

    f32 = mybir.dt.float32
    f16 = mybir.dt.float16
    i32 = mybir.dt.int32
    AF = mybir.ActivationFunctionType
    OP = mybir.AluOpType
    AX = mybir.AxisListType

    x = np.ascontiguousarray(np.asarray(x, dtype=np.float32))
    NB, tpb, tile_ofs, TOT, srcidx, dstrel, qbounds, PAD_ROW = _preprocess_edges(
        np.asarray(edge_index)
    )
    NROWS_TAB = NC * 128

    # ---------------- host-side parameter prep (replicated) ----------------
    rep = lambda v, w: np.ascontiguousarray(
        np.broadcast_to(np.asarray(v, np.float32).reshape(1, w), (128, w))
    )
    w1all = np.ascontiguousarray(
        np.concatenate(
            [np.asarray(conv_W1, np.float32), np.asarray(fin_W1, np.float32)[None]], 0
        )
    )  # [5, 64, 128]
    w2all = np.ascontiguousarray(
        np.concatenate(
            [np.asarray(conv_W2, np.float32), np.asarray(fin_W2, np.float32)[None]], 0
        )
    )  # [5, 128, 64]
    b1rep = np.concatenate(
        [rep(v, H2) for v in list(np.asarray(conv_b1, np.float32)) + [np.asarray(fin_b1)]],
        axis=1,
    )  # [128, 5*128]
    g_fin = np.asarray(fin_bng, np.float32) / np.sqrt(np.float32(1.0 + BN_EPS))
    garr = np.concatenate(
        [rep(v, H2) for v in list(np.asarray(conv_lng, np.float32)) + [g_fin]], axis=1
    )
    barr = np.concatenate(
        [rep(v, H2) for v in list(np.asarray(conv_lnb, np.float32)) + [np.asarray(fin_bnb)]],
        axis=1,
    )
    b2rep = np.concatenate(
        [rep(v, H) for v in list(np.asarray(conv_b2, np.float32)) + [np.asarray(fin_b2)]],
        axis=1,
    )  # [128, 5*64]
    blg = np.asarray(block_lng, np.float32)
    blb = np.asarray(block_lnb, np.float32)
    blkg = np.concatenate([rep(blg[i], H) for i in (1, 2, 3, 0)], axis=1)  # [128, 4*64]
    blkb = np.concatenate([rep(blb[i], H) for i in (1, 2, 3, 0)], axis=1)
    tvals = np.array(
        list(np.asarray(conv_t, np.float32)) + [float(np.asarray(fin_t))], np.float32
    )  # [5]
    tsc = np.ascontiguousarray(np.broadcast_to(tvals.reshape(1, 5), (128, 5)))
    tbi = np.ascontiguousarray(tsc * np.float32(GEN_EPS))
    iota128 = np.ascontiguousarray(
        np.broadcast_to(np.arange(128, dtype=np.float16).reshape(1, 128), (128, 128))
    )
    encW = np.ascontiguousarray(np.asarray(enc_W, np.float32))  # [128, 64]
    encb = rep(enc_b, H)

    # per-core transposed x, packed by (variable-width) blocks
    xT = np.zeros((NC, 128, NB * 128), dtype=np.float32)
    for c in range(NC):
        for b in range(NB):
            q0, q1 = int(qbounds[c, b]), int(qbounds[c, b + 1])
            if q1 > q0:
                xT[c, :, b * 128 : b * 128 + (q1 - q0)] = x[
                    c * NPC + q0 : c * NPC + q1
                ].T

    # ---------------- build the Bass program ----------------
    nc = bass.Bass(dynamic_dma_scratch_size=32768)

    d_xT = nc.dram_tensor("xT", [128, NB * 128], f32, kind="ExternalInput")
    d_idx = nc.dram_tensor("srcidx", [128, TOT], i32, kind="ExternalInput")
    d_drl = nc.dram_tensor("dstrel", [128, TOT], f16, kind="ExternalInput")
    d_w1 = nc.dram_tensor("w1all", [5, H, H2], f32, kind="ExternalInput")
    d_w2 = nc.dram_tensor("w2all", [5, H2, H], f32, kind="ExternalInput")
    d_b1r = nc.dram_tensor("b1rep", [128, 5 * H2], f32, kind="ExternalInput")
    d_gar = nc.dram_tensor("garr", [128, 5 * H2], f32, kind="ExternalInput")
    d_bar = nc.dram_tensor("barr", [128, 5 * H2], f32, kind="ExternalInput")
    d_b2r = nc.dram_tensor("b2rep", [128, 5 * H], f32, kind="ExternalInput")
    d_blkg = nc.dram_tensor("blkg", [128, 4 * H], f32, kind="ExternalInput")
    d_blkb = nc.dram_tensor("blkb", [128, 4 * H], f32, kind="ExternalInput")
    d_tsc = nc.dram_tensor("tsc", [128, 5], f32, kind="ExternalInput")
    d_lneps = nc.dram_tensor("lneps", [128, 1], f32, kind="ExternalInput")
    d_tbi = nc.dram_tensor("tbi", [128, 5], f32, kind="ExternalInput")
    d_iota = nc.dram_tensor("iota", [128, 128], f16, kind="ExternalInput")
    d_encW = nc.dram_tensor("encW", [128, H], f32, kind="ExternalInput")
    d_encb = nc.dram_tensor("encb", [128, H], f32, kind="ExternalInput")
    d_out = nc.dram_tensor("out", [NB * 128, H], f32, kind="ExternalOutput")

    HB = NB // 2  # blocks per half (NB is even)
    C1 = HB * 128  # columns per half
    R2 = NROWS_TAB + 8  # rows per half incl zero-pad rows
    d_Tin_a = nc.dram_tensor("T_in_a", [128, C1], f16)
    d_Tin_b = nc.dram_tensor("T_in_b", [128, C1], f16)
    d_Ttab0 = nc.dram_tensor("T_tab0", [2 * R2, C1], f16, addr_space="Shared")
    d_Ttab1 = nc.dram_tensor("T_tab1", [2 * R2, C1], f16, addr_space="Shared")
    d_tabs = [d_Ttab0, d_Ttab1]
    t_views = [d.rearrange("r (b f) -> (r b) f", f=H2) for d in
               (d_Ttab0[:], d_Ttab1[:])]

    NBH = NB * H  # 3136

    with tile.TileContext(nc) as tc:
        with (
            tc.tile_pool(name="state", bufs=1) as st,
            tc.tile_pool(name="work", bufs=4) as wk,
            tc.tile_pool(name="big", bufs=1) as bg,
            tc.tile_pool(name="wload", bufs=2) as wl,
            tc.tile_pool(name="gat", bufs=48) as gp,
            tc.tile_pool(name="ohp", bufs=24) as ohp,
            tc.tile_pool(name="psum", bufs=4, space="PSUM") as pp,
            tc.tile_pool(name="psum1", bufs=1, space="PSUM") as pq,
        ):
            # persistent state / constants
            idx_sb = st.tile([128, TOT], i32, tag="idx")
            nc.sync.dma_start(out=idx_sb[:], in_=d_idx[:])
            drl_sb = st.tile([128, TOT], f16, tag="drl")
            nc.sync.dma_start(out=drl_sb[:], in_=d_drl[:])
            iota_sb = st.tile([128, 128], f16, tag="iota")
            nc.sync.dma_start(out=iota_sb[:], in_=d_iota[:])
            ident = st.tile([128, 128], f32, tag="ident")
            make_identity(nc, ident[:])
            b1r_sb = st.tile([128, 5 * H2], f32, tag="b1r")
            nc.sync.dma_start(out=b1r_sb[:], in_=d_b1r[:])
            gar_sb = st.tile([128, 5 * H2], f32, tag="gar")
            nc.sync.dma_start(out=gar_sb[:], in_=d_gar[:])
            bar_sb = st.tile([128, 5 * H2], f32, tag="bar")
            nc.sync.dma_start(out=bar_sb[:], in_=d_bar[:])
            b2r_sb = st.tile([128, 5 * H], f32, tag="b2r")
            nc.sync.dma_start(out=b2r_sb[:], in_=d_b2r[:])
            blkg_sb = st.tile([128, 4 * H], f32, tag="blkg")
            nc.sync.dma_start(out=blkg_sb[:], in_=d_blkg[:])
            blkb_sb = st.tile([128, 4 * H], f32, tag="blkb")
            nc.sync.dma_start(out=blkb_sb[:], in_=d_blkb[:])
            tsc_sb = st.tile([128, 5], f32, tag="tsc")
            nc.sync.dma_start(out=tsc_sb[:], in_=d_tsc[:])
            lneps_sb = st.tile([128, 1], f32, tag="lneps")
            nc.sync.dma_start(out=lneps_sb[:], in_=d_lneps[:])
            tbi_sb = st.tile([128, 5], f32, tag="tbi")
            nc.sync.dma_start(out=tbi_sb[:], in_=d_tbi[:])
            encb_sb = st.tile([128, H], f32, tag="encb")
            nc.sync.dma_start(out=encb_sb[:], in_=d_encb[:])
            encW_sb = st.tile([128, H], f32, tag="encW")
            nc.sync.dma_start(out=encW_sb[:], in_=d_encW[:])
            xT_sb = bg.tile([128, NB * 128], f32, tag="bigA")
            nc.sync.dma_start(out=xT_sb[:], in_=d_xT[:])
            tloc_a = st.tile([128, C1], f16, tag="tloca")
            tloc_b = st.tile([128, C1], f16, tag="tlocb")
            tloc_init = tloc_b


            # zero the pad rows of both halves of both T tables (once)
            nc.gpsimd.memset(tloc_init[:], 0)
            for _tab in d_tabs:
                for _h in range(2):
                    nc.sync.dma_start(
                        out=_tab[_h * R2 + NROWS_TAB : (_h + 1) * R2, :],
                        in_=tloc_init[0:8, :],
                    )

            h_a = st.tile([128, NBH], f32, tag="h_a")
            h_b = st.tile([128, NBH], f32, tag="h_b")
            r_sb = st.tile([128, NBH], f32, tag="r_sb")

            # ---------------- encoder: h0 = x @ enc_W + enc_b -> r_sb ------
            for b in range(NB):
                ps_e = pq.tile([128, H], f32, tag="ph2")
                nc.tensor.matmul(
                    out=ps_e[:],
                    lhsT=xT_sb[:, b * 128 : (b + 1) * 128],
                    rhs=encW_sb[:],
                    start=True,
                    stop=True,
                )
                nc.vector.tensor_tensor(
                    out=r_sb[:, b * H : (b + 1) * H],
                    in0=ps_e[:],
                    in1=encb_sb[:],
                    op=OP.add,
                )

            h_cur, h_nxt = h_a, h_b

            def layer_norm_64(dst_ap, src_ap, g_ap, b_ap, relu):
                """dst = [relu](LN(src) * g + b) over 64 feats, node-major."""
                s1 = wk.tile([128, 1], f32, tag="s1")
                nc.vector.reduce_sum(out=s1[:], in_=src_ap, axis=AX.X)
                mu = wk.tile([128, 1], f32, tag="mu")
                nc.vector.tensor_scalar_mul(out=mu[:], in0=s1[:], scalar1=1.0 / H)
                hc = wk.tile([128, H], f32, tag="hc64")
                nc.vector.tensor_scalar_sub(out=hc[:], in0=src_ap, scalar1=mu[:])
                sq = wk.tile([128, H], f32, tag="sq64")
                nc.scalar.square(out=sq[:], in_=hc[:])
                s2 = wk.tile([128, 1], f32, tag="s2")
                nc.vector.reduce_sum(out=s2[:], in_=sq[:], axis=AX.X)
                sd = wk.tile([128, 1], f32, tag="sd")
                nc.scalar.activation(
                    out=sd[:], in_=s2[:], func=AF.Sqrt, bias=lneps_sb[:], scale=1.0 / H
                )
                rstd = wk.tile([128, 1], f32, tag="rstd")
                nc.vector.reciprocal(out=rstd[:], in_=sd[:])
                hn = wk.tile([128, H], f32, tag="hn64")
                nc.vector.tensor_scalar_mul(out=hn[:], in0=hc[:], scalar1=rstd[:])
                hg = wk.tile([128, H], f32, tag="hg64")
                nc.vector.tensor_tensor(out=hg[:], in0=hn[:], in1=g_ap, op=OP.mult)
                if relu:
                    hb_ = wk.tile([128, H], f32, tag="hb64")
                    nc.vector.tensor_tensor(out=hb_[:], in0=hg[:], in1=b_ap, op=OP.add)
                    nc.vector.tensor_scalar_max(out=dst_ap, in0=hb_[:], scalar1=0.0)
                else:
                    nc.vector.tensor_tensor(out=dst_ap, in0=hg[:], in1=b_ap, op=OP.add)

            def t_chunk(b, lidx):
                """tloc[:, b*128:(b+1)*128] = [P2|P1] of r_sb block b, layer lidx."""
                sl = slice(b * H, (b + 1) * H)
                tm = wk.tile([128, H], f32, tag="tm")
                nc.vector.tensor_scalar_max(out=tm[:], in0=r_sb[:, sl], scalar1=0.0)
                tp1 = wk.tile([128, H], f32, tag="tp1")
                nc.scalar.activation(
                    out=tp1[:],
                    in_=tm[:],
                    func=AF.Exp,
                    bias=tbi_sb[:, lidx : lidx + 1],
                    scale=tsc_sb[:, lidx : lidx + 1],
                )
                tme = wk.tile([128, H], f32, tag="tme")
                nc.vector.tensor_scalar_add(out=tme[:], in0=tm[:], scalar1=GEN_EPS)
                tp2 = wk.tile([128, H], f32, tag="tp2")
                nc.vector.tensor_tensor(
                    out=tp2[:], in0=tp1[:], in1=tme[:], op=OP.mult
                )
                if b < HB:
                    _tl, _off = tloc_a, b * H2
                else:
                    _tl, _off = tloc_b, (b - HB) * H2
                nc.vector.tensor_copy(out=_tl[:, _off : _off + H], in_=tp2[:])
                nc.vector.tensor_copy(out=_tl[:, _off + H : _off + H2], in_=tp1[:])

            def push_half1(tab):
                nc.sync.dma_start(out=d_Tin_a[:], in_=tloc_a[:])
                nc.gpsimd.collective_compute(
                    "AllGather",
                    OP.bypass,
                    replica_groups=[list(range(NC))],
                    ins=[d_Tin_a[:]],
                    outs=[tab[0:NROWS_TAB, :]],
                )

            def push_half2(tab):
                nc.sync.dma_start(out=d_Tin_b[:], in_=tloc_b[:])
                nc.gpsimd.collective_compute(
                    "AllGather",
                    OP.bypass,
                    replica_groups=[list(range(NC))],
                    ins=[d_Tin_b[:]],
                    outs=[tab[R2 : R2 + NROWS_TAB, :]],
                )

            for b in range(NB):
                t_chunk(b, 0)
                if b == HB - 1:
                    push_half1(d_tabs[0])
            push_half2(d_tabs[0])

            for l in range(5):
                w1_sb = wl.tile([H, H2], f32, tag="w1")
                nc.sync.dma_start(out=w1_sb[:], in_=d_w1[l])
                w2_sb = wl.tile([H2, H], f32, tag="w2")
                nc.sync.dma_start(out=w2_sb[:], in_=d_w2[l])

                # -------- edge + node phase per dst block ------------------
                for b in range(NB):
                    nt = tpb[b]
                    t0 = tile_ofs[b]
                    ps_agg = pp.tile([128, H2], f32, tag="pagg")
                    for t in range(nt):
                        col = t0 + t
                        g = gp.tile([128, H2], f16, tag="g")
                        nc.gpsimd.indirect_dma_start(
                            out=g[:],
                            out_offset=None,
                            in_=t_views[l % 2],
                            in_offset=IndirectOffsetOnAxis(
                                ap=idx_sb[:, col : col + 1], axis=0
                            ),
                        )
                        oh = ohp.tile([128, 128], f16, tag="oh")
                        nc.vector.tensor_tensor(
                            out=oh[:],
                            in0=iota_sb[:],
                            in1=drl_sb[:, col : col + 1].to_broadcast([128, 128]),
                            op=OP.is_equal,
                        )
                        nc.tensor.matmul(
                            out=ps_agg[:],
                            lhsT=oh[:],
                            rhs=g[:],
                            start=(t == 0),
                            stop=(t == nt - 1),
                        )
                    den = wk.tile([128, H], f32, tag="den")
                    nc.vector.tensor_scalar_max(
                        out=den[:], in0=ps_agg[:, H:H2], scalar1=1e-16
                    )
                    rec = wk.tile([128, H], f32, tag="rec")
                    nc.vector.reciprocal(out=rec[:], in_=den[:])
                    agg = wk.tile([128, H], f32, tag="agg")
                    nc.vector.tensor_tensor(
                        out=agg[:], in0=ps_agg[:, 0:H], in1=rec[:], op=OP.mult
                    )
                    # ---- MLP ----
                    sl = slice(b * H, (b + 1) * H)
                    u = wk.tile([128, H], f32, tag="u")
                    nc.vector.tensor_tensor(
                        out=u[:], in0=agg[:], in1=r_sb[:, sl], op=OP.add
                    )
                    ps_t = pq.tile([H, 128], f32, tag="ptr")
                    nc.tensor.transpose(out=ps_t[:], in_=u[:], identity=ident[:])
                    uT = wk.tile([H, 128], f32, tag="uT")
                    nc.scalar.copy(out=uT[:], in_=ps_t[:])
                    ps_h1 = pq.tile([128, H2], f32, tag="ph1")
                    nc.tensor.matmul(
                        out=ps_h1[:], lhsT=uT[:], rhs=w1_sb[:], start=True, stop=True
                    )
                    l2 = slice(l * H2, (l + 1) * H2)
                    h1 = wk.tile([128, H2], f32, tag="h1")
                    nc.vector.tensor_tensor(
                        out=h1[:], in0=ps_h1[:], in1=b1r_sb[:, l2], op=OP.add
                    )
                    if l < 4:
                        # conv LayerNorm over 128 feats
                        s1 = wk.tile([128, 1], f32, tag="cs1")
                        nc.vector.reduce_sum(out=s1[:], in_=h1[:], axis=AX.X)
                        mu = wk.tile([128, 1], f32, tag="cmu")
                        nc.vector.tensor_scalar_mul(
                            out=mu[:], in0=s1[:], scalar1=1.0 / H2
                        )
                        hc = wk.tile([128, H2], f32, tag="chc")
                        nc.vector.tensor_scalar_sub(out=hc[:], in0=h1[:], scalar1=mu[:])
                        sq = wk.tile([128, H2], f32, tag="csq")
                        nc.scalar.square(out=sq[:], in_=hc[:])
                        s2 = wk.tile([128, 1], f32, tag="cs2")
                        nc.vector.reduce_sum(out=s2[:], in_=sq[:], axis=AX.X)
                        sd = wk.tile([128, 1], f32, tag="csd")
                        nc.scalar.activation(
                            out=sd[:],
                            in_=s2[:],
                            func=AF.Sqrt,
                            bias=lneps_sb[:],
                            scale=1.0 / H2,
                        )
                        rstd = wk.tile([128, 1], f32, tag="crstd")
                        nc.vector.reciprocal(out=rstd[:], in_=sd[:])
                        hn = wk.tile([128, H2], f32, tag="chn")
                        nc.vector.tensor_scalar_mul(
                            out=hn[:], in0=hc[:], scalar1=rstd[:]
                        )
                    else:
                        hn = h1
                    hg = wk.tile([128, H2], f32, tag="chg")
                    nc.vector.tensor_tensor(
                        out=hg[:], in0=hn[:], in1=gar_sb[:, l2], op=OP.mult
                    )
                    hb2 = wk.tile([128, H2], f32, tag="chb")
                    nc.vector.tensor_tensor(
                        out=hb2[:], in0=hg[:], in1=bar_sb[:, l2], op=OP.add
                    )
                    r1 = wk.tile([128, H2], f32, tag="r1")
                    nc.vector.tensor_scalar_max(out=r1[:], in0=hb2[:], scalar1=0.0)
                    ps_t2 = pq.tile([128, 128], f32, tag="ptr2")
                    nc.tensor.transpose(out=ps_t2[:], in_=r1[:], identity=ident[:])
                    r1T = wk.tile([128, 128], f32, tag="r1T")
                    nc.scalar.copy(out=r1T[:], in_=ps_t2[:])
                    ps_h2 = pq.tile([128, H], f32, tag="ph2")
                    nc.tensor.matmul(
                        out=ps_h2[:], lhsT=r1T[:], rhs=w2_sb[:], start=True, stop=True
                    )
                    lh = slice(l * H, (l + 1) * H)
                    if l == 0:
                        nc.vector.tensor_tensor(
                            out=h_nxt[:, sl], in0=ps_h2[:], in1=b2r_sb[:, lh], op=OP.add
                        )
                    elif l < 4:
                        co = wk.tile([128, H], f32, tag="co")
                        nc.vector.tensor_tensor(
                            out=co[:], in0=ps_h2[:], in1=b2r_sb[:, lh], op=OP.add
                        )
                        nc.vector.tensor_tensor(
                            out=h_nxt[:, sl], in0=co[:], in1=h_cur[:, sl], op=OP.add
                        )
                    else:
                        co = wk.tile([128, H], f32, tag="co")
                        nc.vector.tensor_tensor(
                            out=co[:], in0=ps_h2[:], in1=b2r_sb[:, lh], op=OP.add
                        )
                        nc.sync.dma_start(
                            out=d_out[b * 128 : (b + 1) * 128, :], in_=co[:]
                        )
                    if l < 4:
                        # r for conv l+1 from h_nxt, then its T chunk
                        gsl = slice(l * H, (l + 1) * H)
                        layer_norm_64(
                            r_sb[:, sl],
                            h_nxt[:, sl],
                            blkg_sb[:, gsl],
                            blkb_sb[:, gsl],
                            relu=True,
                        )
                        t_chunk(b, l + 1)
                        if b == HB - 1:
                            push_half1(d_tabs[(l + 1) % 2])
                if l < 4:
                    push_half2(d_tabs[(l + 1) % 2])
                    h_cur, h_nxt = h_nxt, h_cur

    in_maps = []
    for c in range(NC):
        in_maps.append(
            {
                "xT": xT[c],
                "srcidx": np.ascontiguousarray(srcidx[c]),
                "dstrel": np.ascontiguousarray(dstrel[c]),
                "w1all": w1all,
                "w2all": w2all,
                "b1rep": b1rep,
                "garr": garr,
                "barr": barr,
                "b2rep": b2rep,
                "blkg": blkg,
                "blkb": blkb,
                "tsc": tsc,
                "lneps": np.full((128, 1), LN_EPS, np.float32),
                "tbi": tbi,
                "iota": iota128,
                "encW": encW,
                "encb": encb,
            }
        )
    res = run_bass_kernel_spmd(nc, in_maps, list(range(NC)), trace=_trace)
    LAST_EXEC_NS = res.exec_time_ns
    out = np.empty((N, H), dtype=np.float32)
    for c in range(NC):
        oc = res.results[c]["out"]
        for b in range(NB):
            q0, q1 = int(qbounds[c, b]), int(qbounds[c, b + 1])
            if q1 > q0:
                out[c * NPC + q0 : c * NPC + q1] = oc[b * 128 : b * 128 + (q1 - q0)]
    return out.astype(np.float32)



# revision 29
# speedup vs baseline: 1.1149x; 1.1149x over previous
"""DeeperGCN forward on 8 Trainium2 NeuronCores (Bass/Tile).

Strategy (graph/data parallel, dst-node sharding):
- Nodes sharded 6250/core. Edges assigned to the core owning their dst,
  sorted by dst, tiled 128/tile within 128-node dst blocks.
- Per conv layer, each core computes node tables P1 = exp(t*msg),
  P2 = P1*msg (msg = relu(r)+eps) for its own nodes; tables are
  AllGathered (fp16) so every core holds the full [N,128] T=[P2|P1].
- Edge phase: indirect-DMA gather of T rows by src, one-hot matmul
  aggregation (onehot[e,dst].T @ T_rows) accumulated in PSUM per dst
  block -> num/den -> agg = num/max(den,1e-16).  This reproduces the
  softmax aggregation exactly (max-subtraction cancels in num/den).
- Node phase: root residual + MLP (Linear->LN/BN->ReLU->Linear) with
  TensorE matmuls/transposes, DVE/ACT elementwise, node-major layout.
"""

import json
import sys
import types

import numpy as np

sys.path.insert(0, "/opt/trn_rl_repo")

# ---------------------------------------------------------------------------
# Workaround: this walrus build supports only ONE semaphore wait per
# instruction; Tile attaches several. Split extras onto NoOp instructions
# at BIR-JSON serialization time.
# ---------------------------------------------------------------------------
_PATCHED = False


def _install_bir_patch():
    global _PATCHED
    if _PATCHED:
        return
    _PATCHED = True
    import concourse.bass as bass

    orig = bass.Bass.to_json_bytes

    def patched(self):
        data = json.loads(orig(self).decode())
        ctr = 0
        for fn in data.get("functions", []):
            for bb in fn.get("blocks", []):
                new_insts = []
                for inst in bb.get("instructions", []):
                    si = inst.get("sync_info")
                    waits = (si or {}).get("on_wait") or []
                    if len(waits) > 1:
                        for w in waits[:-1]:
                            ctr += 1
                            nop = {
                                "engine": inst["engine"],
                                "ins": [],
                                "outs": [],
                                "name": f"{inst['name']}-sw{ctr}",
                                "opcode": "NoOp",
                                "sync_info": {"on_update": [], "on_wait": [w]},
                            }
                            if "debug" in inst:
                                nop["debug"] = inst["debug"]
                            new_insts.append(nop)
                        si["on_wait"] = [waits[-1]]
                    new_insts.append(inst)
                bb["instructions"] = new_insts
        return json.dumps(data).encode()

    bass.Bass.to_json_bytes = patched


def _install_trace_hook():
    """Optional: register the NTFF profiling hook (for test.py timing)."""
    import antenv

    if "antenv.axon_hooks" in sys.modules:
        return
    _m = types.ModuleType("antenv.axon_hooks")
    _m._hook = None
    _m.set_axon_ntff_profile_hook = lambda h: setattr(_m, "_hook", h)
    _m.get_axon_ntff_profile_hook = lambda: _m._hook
    sys.modules["antenv.axon_hooks"] = _m
    antenv.axon_hooks = _m
    try:
        from trn_agent_boot.trn_boot import _ntff_profile_via_ctypes

        _m._hook = _ntff_profile_via_ctypes("/opt/axon/libaxon_pjrt.so")
    except Exception:
        pass


N, NC, NPC = 50000, 8, 6250
H = 64
H2 = 128
F_IN = 128
LN_EPS = 1e-5
BN_EPS = 1e-5
GEN_EPS = 1e-7

LAST_EXEC_NS = None


def _preprocess_edges(edge_index):
    """Per-core dst-sorted edges, variable node-range blocks with a fixed
    2048-edge budget (16 tiles of 128) per block. Returns shared tile
    geometry + per-core index arrays + per-core block node bounds."""
    CAP = 2048
    TPB = CAP // 128  # 16 tiles per block
    src = np.asarray(edge_index[0], dtype=np.int64)
    dst = np.asarray(edge_index[1], dtype=np.int64)
    core_edges = []
    core_qb = []
    for c in range(NC):
        m = (dst >= c * NPC) & (dst < (c + 1) * NPC)
        s_c = src[m]
        d_c = dst[m] - c * NPC
        order = np.argsort(d_c, kind="stable")
        s_c, d_c = s_c[order], d_c[order]
        deg = np.bincount(d_c, minlength=NPC)
        qb = [0]
        nodes_in = 0
        cum = 0
        for n in range(NPC):
            dn = int(deg[n])
            if nodes_in == 128 or cum + dn > CAP:
                qb.append(n)
                nodes_in = 0
                cum = 0
            nodes_in += 1
            cum += dn
        qb.append(NPC)
        core_edges.append((s_c, d_c))
        core_qb.append(qb)
    nblk = max(len(qb) - 1 for qb in core_qb)
    nblk += nblk % 2  # even block count for half-split AllGather
    # pad bounds to nblk+1 entries (trailing empty blocks)
    qbounds = np.full((NC, nblk + 1), NPC, dtype=np.int64)
    for c in range(NC):
        qb = core_qb[c]
        qbounds[c, : len(qb)] = qb
    # node -> (block, pos) per core
    blk_of = np.zeros((NC, NPC), dtype=np.int64)
    pos_of = np.zeros((NC, NPC), dtype=np.int64)
    for c in range(NC):
        for b in range(nblk):
            q0, q1 = int(qbounds[c, b]), int(qbounds[c, b + 1])
            if q1 > q0:
                blk_of[c, q0:q1] = b
                pos_of[c, q0:q1] = np.arange(q1 - q0)
    R2 = NC * 128 + 8  # rows per half-table incl zero-pad rows
    HBK = nblk // 2
    assert HBK <= 31, f"int16 gather index overflow: HBK={HBK}"
    pad_local = (NC * 128) * HBK  # zeroed pad row, relative to half base
    # split each block's edges by src table-half; per-half tile counts are
    # cross-core maxes so all cores share the program structure
    half_lists = [[None] * (2 * nblk) for _ in range(NC)]
    for c in range(NC):
        s_c, d_c = core_edges[c]
        bounds = np.searchsorted(d_c, qbounds[c])
        cs = s_c // NPC
        ls = s_c % NPC
        bk = blk_of[cs, ls]
        hh = bk // HBK
        loc = (cs * 128 + pos_of[cs, ls]) * HBK + (bk % HBK)
        for b in range(nblk):
            e0, e1 = int(bounds[b]), int(bounds[b + 1])
            dp = d_c[e0:e1] - qbounds[c, b]
            for h in (0, 1):
                sel = hh[e0:e1] == h
                half_lists[c][b * 2 + h] = (
                    loc[e0:e1][sel].astype(np.int64),
                    dp[sel].astype(np.int64),
                )
    ntl = np.zeros(nblk, dtype=np.int64)
    nth = np.zeros(nblk, dtype=np.int64)
    for b in range(nblk):
        for c in range(NC):
            ntl[b] = max(ntl[b], -(-len(half_lists[c][b * 2][0]) // 128))
            nth[b] = max(nth[b], -(-len(half_lists[c][b * 2 + 1][0]) // 128))
    ntl[(ntl + nth) == 0] = 1  # every block needs >=1 tile (PSUM is read)
    tpb = ntl + nth
    tile_ofs = np.concatenate([[0], np.cumsum(tpb)])
    tot = int(tile_ofs[-1])
    SLOT_TOT = tot * 8
    slot_of = np.zeros((nblk, 2), dtype=np.int64)
    s = 0
    for b in range(nblk):
        slot_of[b, 0] = s
        s += int(ntl[b]) * 8
        slot_of[b, 1] = s
        s += int(nth[b]) * 8
    idxs = np.zeros((NC, 128, SLOT_TOT), dtype=np.int16)
    dstrel = np.full((NC, 128, tot), 255.0, dtype=np.float16)
    qq = np.arange(128)
    for c in range(NC):
        for b in range(nblk):
            t0 = int(tile_ofs[b])
            for h, nt in ((0, int(ntl[b])), (1, int(nth[b]))):
                if nt == 0:
                    continue
                loc, dp = half_lists[c][b * 2 + h]
                m = nt * 128
                L = np.full(m, pad_local, dtype=np.int64)
                L[: len(loc)] = loc
                s0 = int(slot_of[b, h])
                wr = L.reshape(m // 16, 16).T
                idxs[c, :, s0 : s0 + m // 16] = wr[qq % 16, :]
                dpp = np.full(m, 255, dtype=np.int64)
                dpp[: len(dp)] = dp
                dstrel[c, :, t0 : t0 + nt] = (
                    dpp.reshape(nt, 128).T.astype(np.float16)
                )
                t0 += nt
    return (
        nblk,
        ntl.astype(int).tolist(),
        nth.astype(int).tolist(),
        tile_ofs.astype(int).tolist(),
        tot,
        SLOT_TOT,
        slot_of,
        idxs,
        dstrel,
        qbounds,
    )


def kernel(
    x,
    edge_index,
    enc_W,
    enc_b,
    conv_t,
    conv_W1,
    conv_b1,
    conv_lng,
    conv_lnb,
    conv_W2,
    conv_b2,
    block_lng,
    block_lnb,
    fin_t,
    fin_W1,
    fin_b1,
    fin_bng,
    fin_bnb,
    fin_W2,
    fin_b2,
    _trace=False,
):
    global LAST_EXEC_NS
    _install_bir_patch()
    if _trace:
        _install_trace_hook()

    import concourse.bass as bass
    import concourse.mybir as mybir
    import concourse.tile as tile
    from concourse.bass import IndirectOffsetOnAxis
    from concourse.bass_utils import run_bass_kernel_spmd
    from concourse.masks import make_identity

    f32 = mybir.dt.float32
    f16 = mybir.dt.float16
    i32 = mybir.dt.int32
    AF = mybir.ActivationFunctionType
    OP = mybir.AluOpType
    AX = mybir.AxisListType

    from concourse import library_config
    from concourse.library_overlay import lower_extended_insts

    x = np.ascontiguousarray(np.asarray(x, dtype=np.float32))
    (NB, ntl, nth, tile_ofs, TOT, SLOT_TOT, slot_of, idxs_np, dstrel, qbounds) = (
        _preprocess_edges(np.asarray(edge_index))
    )
    NROWS_TAB = NC * 128
    ROWS_HALF = (NROWS_TAB + 8) * (NB // 2)
    i16 = mybir.dt.int16

    # ---------------- host-side parameter prep (replicated) ----------------
    rep = lambda v, w: np.ascontiguousarray(
        np.broadcast_to(np.asarray(v, np.float32).reshape(1, w), (128, w))
    )
    w1all = np.ascontiguousarray(
        np.concatenate(
            [np.asarray(conv_W1, np.float32), np.asarray(fin_W1, np.float32)[None]], 0
        )
    )  # [5, 64, 128]
    w2all = np.ascontiguousarray(
        np.concatenate(
            [np.asarray(conv_W2, np.float32), np.asarray(fin_W2, np.float32)[None]], 0
        )
    )  # [5, 128, 64]
    b1rep = np.concatenate(
        [rep(v, H2) for v in list(np.asarray(conv_b1, np.float32)) + [np.asarray(fin_b1)]],
        axis=1,
    )  # [128, 5*128]
    g_fin = np.asarray(fin_bng, np.float32) / np.sqrt(np.float32(1.0 + BN_EPS))
    garr = np.concatenate(
        [rep(v, H2) for v in list(np.asarray(conv_lng, np.float32)) + [g_fin]], axis=1
    )
    barr = np.concatenate(
        [rep(v, H2) for v in list(np.asarray(conv_lnb, np.float32)) + [np.asarray(fin_bnb)]],
        axis=1,
    )
    b2rep = np.concatenate(
        [rep(v, H) for v in list(np.asarray(conv_b2, np.float32)) + [np.asarray(fin_b2)]],
        axis=1,
    )  # [128, 5*64]
    blg = np.asarray(block_lng, np.float32)
    blb = np.asarray(block_lnb, np.float32)
    blkg = np.concatenate([rep(blg[i], H) for i in (1, 2, 3, 0)], axis=1)  # [128, 4*64]
    blkb = np.concatenate([rep(blb[i], H) for i in (1, 2, 3, 0)], axis=1)
    tvals = np.array(
        list(np.asarray(conv_t, np.float32)) + [float(np.asarray(fin_t))], np.float32
    )  # [5]
    tsc = np.ascontiguousarray(np.broadcast_to(tvals.reshape(1, 5), (128, 5)))
    tbi = np.ascontiguousarray(tsc * np.float32(GEN_EPS))
    iota128 = np.ascontiguousarray(
        np.broadcast_to(np.arange(128, dtype=np.float16).reshape(1, 128), (128, 128))
    )
    encW = np.ascontiguousarray(np.asarray(enc_W, np.float32))  # [128, 64]
    encb = rep(enc_b, H)

    # per-core transposed x, packed by (variable-width) blocks
    xT = np.zeros((NC, 128, NB * 128), dtype=np.float32)
    for c in range(NC):
        for b in range(NB):
            q0, q1 = int(qbounds[c, b]), int(qbounds[c, b + 1])
            if q1 > q0:
                xT[c, :, b * 128 : b * 128 + (q1 - q0)] = x[
                    c * NPC + q0 : c * NPC + q1
                ].T

    # ---------------- build the Bass program ----------------
    nc = bass.Bass(dynamic_dma_scratch_size=32768)

    d_xT = nc.dram_tensor("xT", [128, NB * 128], f32, kind="ExternalInput")
    d_idx = nc.dram_tensor("idxs", [128, SLOT_TOT], i16, kind="ExternalInput")
    d_drl = nc.dram_tensor("dstrel", [128, TOT], f16, kind="ExternalInput")
    d_w1 = nc.dram_tensor("w1all", [5, H, H2], f32, kind="ExternalInput")
    d_w2 = nc.dram_tensor("w2all", [5, H2, H], f32, kind="ExternalInput")
    d_b1r = nc.dram_tensor("b1rep", [128, 5 * H2], f32, kind="ExternalInput")
    d_gar = nc.dram_tensor("garr", [128, 5 * H2], f32, kind="ExternalInput")
    d_bar = nc.dram_tensor("barr", [128, 5 * H2], f32, kind="ExternalInput")
    d_b2r = nc.dram_tensor("b2rep", [128, 5 * H], f32, kind="ExternalInput")
    d_blkg = nc.dram_tensor("blkg", [128, 4 * H], f32, kind="ExternalInput")
    d_blkb = nc.dram_tensor("blkb", [128, 4 * H], f32, kind="ExternalInput")
    d_tsc = nc.dram_tensor("tsc", [128, 5], f32, kind="ExternalInput")
    d_lneps = nc.dram_tensor("lneps", [128, 1], f32, kind="ExternalInput")
    d_tbi = nc.dram_tensor("tbi", [128, 5], f32, kind="ExternalInput")
    d_iota = nc.dram_tensor("iota", [128, 128], f16, kind="ExternalInput")
    d_encW = nc.dram_tensor("encW", [128, H], f32, kind="ExternalInput")
    d_encb = nc.dram_tensor("encb", [128, H], f32, kind="ExternalInput")
    d_out = nc.dram_tensor("out", [NB * 128, H], f32, kind="ExternalOutput")

    HB = NB // 2  # blocks per half (NB is even)
    C1 = HB * 128  # columns per half
    R2 = NROWS_TAB + 8  # rows per half incl zero-pad rows
    d_Tin_a = nc.dram_tensor("T_in_a", [128, C1], f16)
    d_Tin_b = nc.dram_tensor("T_in_b", [128, C1], f16)
    d_Ttab0 = nc.dram_tensor("T_tab0", [2 * R2, C1], f16, addr_space="Shared")
    d_Ttab1 = nc.dram_tensor("T_tab1", [2 * R2, C1], f16, addr_space="Shared")
    d_tabs = [d_Ttab0, d_Ttab1]
    t_half = []
    for d in (d_Ttab0[:], d_Ttab1[:]):
        full = d.rearrange("r (b f) -> (r b) f", f=H2)
        t_half.append([full[0:ROWS_HALF, :], full[ROWS_HALF : 2 * ROWS_HALF, :]])

    NBH = NB * H  # 3136

    with tile.TileContext(nc) as tc:
        with (
            tc.tile_pool(name="state", bufs=1) as st,
            tc.tile_pool(name="work", bufs=4) as wk,
            tc.tile_pool(name="big", bufs=1) as bg,
            tc.tile_pool(name="wload", bufs=2) as wl,
            tc.tile_pool(name="gat", bufs=3) as gp,
            tc.tile_pool(name="ohp", bufs=24) as ohp,
            tc.tile_pool(name="psum", bufs=4, space="PSUM") as pp,
            tc.tile_pool(name="psum1", bufs=1, space="PSUM") as pq,
        ):
            # persistent state / constants
            idx_sb = st.tile([128, SLOT_TOT], i16, tag="idx")
            nc.sync.dma_start(out=idx_sb[:], in_=d_idx[:])
            drl_sb = st.tile([128, TOT], f16, tag="drl")
            nc.sync.dma_start(out=drl_sb[:], in_=d_drl[:])
            iota_sb = st.tile([128, 128], f16, tag="iota")
            nc.sync.dma_start(out=iota_sb[:], in_=d_iota[:])
            ident = st.tile([128, 128], f32, tag="ident")
            make_identity(nc, ident[:])
            b1r_sb = st.tile([128, 5 * H2], f32, tag="b1r")
            nc.sync.dma_start(out=b1r_sb[:], in_=d_b1r[:])
            gar_sb = st.tile([128, 5 * H2], f32, tag="gar")
            nc.sync.dma_start(out=gar_sb[:], in_=d_gar[:])
            bar_sb = st.tile([128, 5 * H2], f32, tag="bar")
            nc.sync.dma_start(out=bar_sb[:], in_=d_bar[:])
            b2r_sb = st.tile([128, 5 * H], f32, tag="b2r")
            nc.sync.dma_start(out=b2r_sb[:], in_=d_b2r[:])
            blkg_sb = st.tile([128, 4 * H], f32, tag="blkg")
            nc.sync.dma_start(out=blkg_sb[:], in_=d_blkg[:])
            blkb_sb = st.tile([128, 4 * H], f32, tag="blkb")
            nc.sync.dma_start(out=blkb_sb[:], in_=d_blkb[:])
            tsc_sb = st.tile([128, 5], f32, tag="tsc")
            nc.sync.dma_start(out=tsc_sb[:], in_=d_tsc[:])
            lneps_sb = st.tile([128, 1], f32, tag="lneps")
            nc.sync.dma_start(out=lneps_sb[:], in_=d_lneps[:])
            tbi_sb = st.tile([128, 5], f32, tag="tbi")
            nc.sync.dma_start(out=tbi_sb[:], in_=d_tbi[:])
            encb_sb = st.tile([128, H], f32, tag="encb")
            nc.sync.dma_start(out=encb_sb[:], in_=d_encb[:])
            encW_sb = st.tile([128, H], f32, tag="encW")
            nc.sync.dma_start(out=encW_sb[:], in_=d_encW[:])
            xT_sb = bg.tile([128, NB * 128], f32, tag="bigA")
            nc.sync.dma_start(out=xT_sb[:], in_=d_xT[:])
            tloc_a = st.tile([128, C1], f16, tag="tloca")
            tloc_b = st.tile([128, C1], f16, tag="tlocb")
            tloc_init = tloc_b


            # zero the pad rows of both halves of both T tables (once)
            nc.gpsimd.memset(tloc_init[:], 0)
            for _tab in d_tabs:
                for _h in range(2):
                    nc.sync.dma_start(
                        out=_tab[_h * R2 + NROWS_TAB : (_h + 1) * R2, :],
                        in_=tloc_init[0:8, :],
                    )

            nc.gpsimd.load_library(library_config.mlp)
            _nt_regs = {}
            for _nt in range(1, 9):
                _nt_regs[_nt] = nc.gpsimd.to_reg(_nt * 128)

            h_a = st.tile([128, NBH], f32, tag="h_a")
            h_b = st.tile([128, NBH], f32, tag="h_b")
            r_sb = st.tile([128, NBH], f32, tag="r_sb")

            # ---------------- encoder: h0 = x @ enc_W + enc_b -> r_sb ------
            for b in range(NB):
                ps_e = pq.tile([128, H], f32, tag="ph2")
                nc.tensor.matmul(
                    out=ps_e[:],
                    lhsT=xT_sb[:, b * 128 : (b + 1) * 128],
                    rhs=encW_sb[:],
                    start=True,
                    stop=True,
                )
                nc.vector.tensor_tensor(
                    out=r_sb[:, b * H : (b + 1) * H],
                    in0=ps_e[:],
                    in1=encb_sb[:],
                    op=OP.add,
                )

            h_cur, h_nxt = h_a, h_b

            def layer_norm_64(dst_ap, src_ap, g_ap, b_ap, relu):
                """dst = [relu](LN(src) * g + b) over 64 feats, node-major."""
                s1 = wk.tile([128, 1], f32, tag="s1")
                nc.vector.reduce_sum(out=s1[:], in_=src_ap, axis=AX.X)
                mu = wk.tile([128, 1], f32, tag="mu")
                nc.vector.tensor_scalar_mul(out=mu[:], in0=s1[:], scalar1=1.0 / H)
                hc = wk.tile([128, H], f32, tag="hc64")
                nc.vector.tensor_scalar_sub(out=hc[:], in0=src_ap, scalar1=mu[:])
                sq = wk.tile([128, H], f32, tag="sq64")
                nc.scalar.square(out=sq[:], in_=hc[:])
                s2 = wk.tile([128, 1], f32, tag="s2")
                nc.vector.reduce_sum(out=s2[:], in_=sq[:], axis=AX.X)
                sd = wk.tile([128, 1], f32, tag="sd")
                nc.scalar.activation(
                    out=sd[:], in_=s2[:], func=AF.Sqrt, bias=lneps_sb[:], scale=1.0 / H
                )
                rstd = wk.tile([128, 1], f32, tag="rstd")
                nc.vector.reciprocal(out=rstd[:], in_=sd[:])
                hn = wk.tile([128, H], f32, tag="hn64")
                nc.vector.tensor_scalar_mul(out=hn[:], in0=hc[:], scalar1=rstd[:])
                hg = wk.tile([128, H], f32, tag="hg64")
                nc.vector.tensor_tensor(out=hg[:], in0=hn[:], in1=g_ap, op=OP.mult)
                if relu:
                    hb_ = wk.tile([128, H], f32, tag="hb64")
                    nc.vector.tensor_tensor(out=hb_[:], in0=hg[:], in1=b_ap, op=OP.add)
                    nc.vector.tensor_scalar_max(out=dst_ap, in0=hb_[:], scalar1=0.0)
                else:
                    nc.vector.tensor_tensor(out=dst_ap, in0=hg[:], in1=b_ap, op=OP.add)

            def t_chunk(b, lidx):
                """tloc[:, b*128:(b+1)*128] = [P2|P1] of r_sb block b, layer lidx."""
                sl = slice(b * H, (b + 1) * H)
                tm = wk.tile([128, H], f32, tag="tm")
                nc.vector.tensor_scalar_max(out=tm[:], in0=r_sb[:, sl], scalar1=0.0)
                tp1 = wk.tile([128, H], f32, tag="tp1")
                nc.scalar.activation(
                    out=tp1[:],
                    in_=tm[:],
                    func=AF.Exp,
                    bias=tbi_sb[:, lidx : lidx + 1],
                    scale=tsc_sb[:, lidx : lidx + 1],
                )
                tme = wk.tile([128, H], f32, tag="tme")
                nc.vector.tensor_scalar_add(out=tme[:], in0=tm[:], scalar1=GEN_EPS)
                tp2 = wk.tile([128, H], f32, tag="tp2")
                nc.vector.tensor_tensor(
                    out=tp2[:], in0=tp1[:], in1=tme[:], op=OP.mult
                )
                if b < HB:
                    _tl, _off = tloc_a, b * H2
                else:
                    _tl, _off = tloc_b, (b - HB) * H2
                nc.vector.tensor_copy(out=_tl[:, _off : _off + H], in_=tp2[:])
                nc.vector.tensor_copy(out=_tl[:, _off + H : _off + H2], in_=tp1[:])

            def push_half1(tab):
                nc.sync.dma_start(out=d_Tin_a[:], in_=tloc_a[:])
                nc.gpsimd.collective_compute(
                    "AllGather",
                    OP.bypass,
                    replica_groups=[list(range(NC))],
                    ins=[d_Tin_a[:]],
                    outs=[tab[0:NROWS_TAB, :]],
                )

            def push_half2(tab):
                nc.sync.dma_start(out=d_Tin_b[:], in_=tloc_b[:])
                nc.gpsimd.collective_compute(
                    "AllGather",
                    OP.bypass,
                    replica_groups=[list(range(NC))],
                    ins=[d_Tin_b[:]],
                    outs=[tab[R2 : R2 + NROWS_TAB, :]],
                )

            for b in range(NB):
                t_chunk(b, 0)
                if b == HB - 1:
                    push_half1(d_tabs[0])
            push_half2(d_tabs[0])

            for l in range(5):
                w1_sb = wl.tile([H, H2], f32, tag="w1")
                nc.sync.dma_start(out=w1_sb[:], in_=d_w1[l])
                w2_sb = wl.tile([H2, H], f32, tag="w2")
                nc.sync.dma_start(out=w2_sb[:], in_=d_w2[l])

                # -------- edge + node phase per dst block ------------------
                for b in range(NB):
                    nl, nh = int(ntl[b]), int(nth[b])
                    nt_all = nl + nh
                    t0 = tile_ofs[b]
                    ps_agg = pp.tile([128, H2], f32, tag="pagg")
                    gts = []
                    for hh, nt in ((0, nl), (1, nh)):
                        if nt == 0:
                            continue
                        g_t = gp.tile([128, 16 * H2], f16, tag=f"g{hh}")
                        s0 = int(slot_of[b, hh])
                        # dma_gather breaks above 1024 idxs/instruction:
                        # split into <=8-tile chunks
                        for c0 in range(0, nt, 8):
                            cn = min(8, nt - c0)
                            g3v = g_t[
                                :, c0 * H2 : (c0 + cn) * H2
                            ].rearrange("p (t f) -> p t f", f=H2)
                            nc.gpsimd.dma_gather(
                                g3v,
                                t_half[l % 2][hh],
                                idx_sb[:, s0 + c0 * 8 : s0 + (c0 + cn) * 8],
                                cn * 128,
                                _nt_regs[cn],
                                H2,
                            )
                        gts.append((g_t, nt))
                    ti = 0
                    for g_t, nt in gts:
                        for t in range(nt):
                            col = t0 + ti
                            oh = ohp.tile([128, 128], f16, tag="oh")
                            nc.vector.tensor_tensor(
                                out=oh[:],
                                in0=iota_sb[:],
                                in1=drl_sb[:, col : col + 1].to_broadcast([128, 128]),
                                op=OP.is_equal,
                            )
                            nc.tensor.matmul(
                                out=ps_agg[:],
                                lhsT=oh[:],
                                rhs=g_t[:, t * H2 : (t + 1) * H2],
                                start=(ti == 0),
                                stop=(ti == nt_all - 1),
                            )
                            ti += 1
                    den = wk.tile([128, H], f32, tag="den")
                    nc.vector.tensor_scalar_max(
                        out=den[:], in0=ps_agg[:, H:H2], scalar1=1e-16
                    )
                    rec = wk.tile([128, H], f32, tag="rec")
                    nc.vector.reciprocal(out=rec[:], in_=den[:])
                    agg = wk.tile([128, H], f32, tag="agg")
                    nc.vector.tensor_tensor(
                        out=agg[:], in0=ps_agg[:, 0:H], in1=rec[:], op=OP.mult
                    )
                    # ---- MLP ----
                    sl = slice(b * H, (b + 1) * H)
                    u = wk.tile([128, H], f32, tag="u")
                    nc.vector.tensor_tensor(
                        out=u[:], in0=agg[:], in1=r_sb[:, sl], op=OP.add
                    )
                    ps_t = pq.tile([H, 128], f32, tag="ptr")
                    nc.tensor.transpose(out=ps_t[:], in_=u[:], identity=ident[:])
                    uT = wk.tile([H, 128], f32, tag="uT")
                    nc.scalar.copy(out=uT[:], in_=ps_t[:])
                    ps_h1 = pq.tile([128, H2], f32, tag="ph1")
                    nc.tensor.matmul(
                        out=ps_h1[:], lhsT=uT[:], rhs=w1_sb[:], start=True, stop=True
                    )
                    l2 = slice(l * H2, (l + 1) * H2)
                    h1 = wk.tile([128, H2], f32, tag="h1")
                    nc.vector.tensor_tensor(
                        out=h1[:], in0=ps_h1[:], in1=b1r_sb[:, l2], op=OP.add
                    )
                    if l < 4:
                        # conv LayerNorm over 128 feats
                        s1 = wk.tile([128, 1], f32, tag="cs1")
                        nc.vector.reduce_sum(out=s1[:], in_=h1[:], axis=AX.X)
                        mu = wk.tile([128, 1], f32, tag="cmu")
                        nc.vector.tensor_scalar_mul(
                            out=mu[:], in0=s1[:], scalar1=1.0 / H2
                        )
                        hc = wk.tile([128, H2], f32, tag="chc")
                        nc.vector.tensor_scalar_sub(out=hc[:], in0=h1[:], scalar1=mu[:])
                        sq = wk.tile([128, H2], f32, tag="csq")
                        nc.scalar.square(out=sq[:], in_=hc[:])
                        s2 = wk.tile([128, 1], f32, tag="cs2")
                        nc.vector.reduce_sum(out=s2[:], in_=sq[:], axis=AX.X)
                        sd = wk.tile([128, 1], f32, tag="csd")
                        nc.scalar.activation(
                            out=sd[:],
                            in_=s2[:],
                            func=AF.Sqrt,
                            bias=lneps_sb[:],
                            scale=1.0 / H2,
                        )
                        rstd = wk.tile([128, 1], f32, tag="crstd")
                        nc.vector.reciprocal(out=rstd[:], in_=sd[:])
                        hn = wk.tile([128, H2], f32, tag="chn")
                        nc.vector.tensor_scalar_mul(
                            out=hn[:], in0=hc[:], scalar1=rstd[:]
                        )
                    else:
                        hn = h1
                    hg = wk.tile([128, H2], f32, tag="chg")
                    nc.vector.tensor_tensor(
                        out=hg[:], in0=hn[:], in1=gar_sb[:, l2], op=OP.mult
                    )
                    hb2 = wk.tile([128, H2], f32, tag="chb")
                    nc.vector.tensor_tensor(
                        out=hb2[:], in0=hg[:], in1=bar_sb[:, l2], op=OP.add
                    )
                    r1 = wk.tile([128, H2], f32, tag="r1")
                    nc.vector.tensor_scalar_max(out=r1[:], in0=hb2[:], scalar1=0.0)
                    ps_t2 = pq.tile([128, 128], f32, tag="ptr2")
                    nc.tensor.transpose(out=ps_t2[:], in_=r1[:], identity=ident[:])
                    r1T = wk.tile([128, 128], f32, tag="r1T")
                    nc.scalar.copy(out=r1T[:], in_=ps_t2[:])
                    ps_h2 = pq.tile([128, H], f32, tag="ph2")
                    nc.tensor.matmul(
                        out=ps_h2[:], lhsT=r1T[:], rhs=w2_sb[:], start=True, stop=True
                    )
                    lh = slice(l * H, (l + 1) * H)
                    if l == 0:
                        nc.vector.tensor_tensor(
                            out=h_nxt[:, sl], in0=ps_h2[:], in1=b2r_sb[:, lh], op=OP.add
                        )
                    elif l < 4:
                        co = wk.tile([128, H], f32, tag="co")
                        nc.vector.tensor_tensor(
                            out=co[:], in0=ps_h2[:], in1=b2r_sb[:, lh], op=OP.add
                        )
                        nc.vector.tensor_tensor(
                            out=h_nxt[:, sl], in0=co[:], in1=h_cur[:, sl], op=OP.add
                        )
                    else:
                        co = wk.tile([128, H], f32, tag="co")
                        nc.vector.tensor_tensor(
                            out=co[:], in0=ps_h2[:], in1=b2r_sb[:, lh], op=OP.add
                        )
                        nc.sync.dma_start(
                            out=d_out[b * 128 : (b + 1) * 128, :], in_=co[:]
                        )
                    if l < 4:
                        # r for conv l+1 from h_nxt, then its T chunk
                        gsl = slice(l * H, (l + 1) * H)
                        layer_norm_64(
                            r_sb[:, sl],
                            h_nxt[:, sl],
                            blkg_sb[:, gsl],
                            blkb_sb[:, gsl],
                            relu=True,
                        )
                        t_chunk(b, l + 1)
                        if b == HB - 1:
                            push_half1(d_tabs[(l + 1) % 2])
                if l < 4:
                    push_half2(d_tabs[(l + 1) % 2])
                    h_cur, h_nxt = h_nxt, h_cur

    in_maps = []
    for c in range(NC):
        in_maps.append(
            {
                "xT": xT[c],
                "idxs": np.ascontiguousarray(idxs_np[c]),
                "dstrel": np.ascontiguousarray(dstrel[c]),
                "w1all": w1all,
                "w2all": w2all,
                "b1rep": b1rep,
                "garr": garr,
                "barr": barr,
                "b2rep": b2rep,
                "blkg": blkg,
                "blkb": blkb,
                "tsc": tsc,
                "lneps": np.full((128, 1), LN_EPS, np.float32),
                "tbi": tbi,
                "iota": iota128,
                "encW": encW,
                "encb": encb,
            }
        )
    lower_extended_insts(nc)
    res = run_bass_kernel_spmd(nc, in_maps, list(range(NC)), trace=_trace)
    LAST_EXEC_NS = res.exec_time_ns
    out = np.empty((N, H), dtype=np.float32)
    for c in range(NC):
        oc = res.results[c]["out"]
        for b in range(NB):
            q0, q1 = int(qbounds[c, b]), int(qbounds[c, b + 1])
            if q1 > q0:
                out[c * NPC + q0 : c * NPC + q1] = oc[b * 128 : b * 128 + (q1 - q0)]
    return out.astype(np.float32)



# revision 32
# speedup vs baseline: 1.2123x; 1.0874x over previous
"""DeeperGCN forward on 8 Trainium2 NeuronCores (Bass/Tile).

Strategy (graph/data parallel, dst-node sharding):
- Nodes sharded 6250/core. Edges assigned to the core owning their dst,
  sorted by dst, tiled 128/tile within 128-node dst blocks.
- Per conv layer, each core computes node tables P1 = exp(t*msg),
  P2 = P1*msg (msg = relu(r)+eps) for its own nodes; tables are
  AllGathered (fp16) so every core holds the full [N,128] T=[P2|P1].
- Edge phase: indirect-DMA gather of T rows by src, one-hot matmul
  aggregation (onehot[e,dst].T @ T_rows) accumulated in PSUM per dst
  block -> num/den -> agg = num/max(den,1e-16).  This reproduces the
  softmax aggregation exactly (max-subtraction cancels in num/den).
- Node phase: root residual + MLP (Linear->LN/BN->ReLU->Linear) with
  TensorE matmuls/transposes, DVE/ACT elementwise, node-major layout.
"""

import json
import sys
import types

import numpy as np

sys.path.insert(0, "/opt/trn_rl_repo")

# ---------------------------------------------------------------------------
# Workaround: this walrus build supports only ONE semaphore wait per
# instruction; Tile attaches several. Split extras onto NoOp instructions
# at BIR-JSON serialization time.
# ---------------------------------------------------------------------------
_PATCHED = False


def _install_bir_patch():
    global _PATCHED
    if _PATCHED:
        return
    _PATCHED = True
    import concourse.bass as bass

    orig = bass.Bass.to_json_bytes

    def patched(self):
        data = json.loads(orig(self).decode())
        ctr = 0
        for fn in data.get("functions", []):
            for bb in fn.get("blocks", []):
                new_insts = []
                for inst in bb.get("instructions", []):
                    si = inst.get("sync_info")
                    waits = (si or {}).get("on_wait") or []
                    if len(waits) > 1:
                        for w in waits[:-1]:
                            ctr += 1
                            nop = {
                                "engine": inst["engine"],
                                "ins": [],
                                "outs": [],
                                "name": f"{inst['name']}-sw{ctr}",
                                "opcode": "NoOp",
                                "sync_info": {"on_update": [], "on_wait": [w]},
                            }
                            if "debug" in inst:
                                nop["debug"] = inst["debug"]
                            new_insts.append(nop)
                        si["on_wait"] = [waits[-1]]
                    new_insts.append(inst)
                bb["instructions"] = new_insts
        return json.dumps(data).encode()

    bass.Bass.to_json_bytes = patched


def _install_trace_hook():
    """Optional: register the NTFF profiling hook (for test.py timing)."""
    import antenv

    if "antenv.axon_hooks" in sys.modules:
        return
    _m = types.ModuleType("antenv.axon_hooks")
    _m._hook = None
    _m.set_axon_ntff_profile_hook = lambda h: setattr(_m, "_hook", h)
    _m.get_axon_ntff_profile_hook = lambda: _m._hook
    sys.modules["antenv.axon_hooks"] = _m
    antenv.axon_hooks = _m
    try:
        from trn_agent_boot.trn_boot import _ntff_profile_via_ctypes

        _m._hook = _ntff_profile_via_ctypes("/opt/axon/libaxon_pjrt.so")
    except Exception:
        pass


N, NC, NPC = 50000, 8, 6250
H = 64
H2 = 128
F_IN = 128
LN_EPS = 1e-5
BN_EPS = 1e-5
GEN_EPS = 1e-7

LAST_EXEC_NS = None


def _preprocess_edges(edge_index):
    """Per-core dst-sorted edges, variable node-range blocks with a fixed
    2048-edge budget (16 tiles of 128) per block. Returns shared tile
    geometry + per-core index arrays + per-core block node bounds."""
    CAP = 2048
    TPB = CAP // 128  # 16 tiles per block
    src = np.asarray(edge_index[0], dtype=np.int64)
    dst = np.asarray(edge_index[1], dtype=np.int64)
    core_edges = []
    core_qb = []
    for c in range(NC):
        m = (dst >= c * NPC) & (dst < (c + 1) * NPC)
        s_c = src[m]
        d_c = dst[m] - c * NPC
        order = np.argsort(d_c, kind="stable")
        s_c, d_c = s_c[order], d_c[order]
        deg = np.bincount(d_c, minlength=NPC)
        qb = [0]
        nodes_in = 0
        cum = 0
        for n in range(NPC):
            dn = int(deg[n])
            if nodes_in == 128 or cum + dn > CAP:
                qb.append(n)
                nodes_in = 0
                cum = 0
            nodes_in += 1
            cum += dn
        qb.append(NPC)
        core_edges.append((s_c, d_c))
        core_qb.append(qb)
    nblk = max(len(qb) - 1 for qb in core_qb)
    nblk += nblk % 2  # even block count for half-split AllGather
    # pad bounds to nblk+1 entries (trailing empty blocks)
    qbounds = np.full((NC, nblk + 1), NPC, dtype=np.int64)
    for c in range(NC):
        qb = core_qb[c]
        qbounds[c, : len(qb)] = qb
    # node -> (block, pos) per core
    blk_of = np.zeros((NC, NPC), dtype=np.int64)
    pos_of = np.zeros((NC, NPC), dtype=np.int64)
    for c in range(NC):
        for b in range(nblk):
            q0, q1 = int(qbounds[c, b]), int(qbounds[c, b + 1])
            if q1 > q0:
                blk_of[c, q0:q1] = b
                pos_of[c, q0:q1] = np.arange(q1 - q0)
    R2 = NC * 128 + 8  # rows per half-table incl zero-pad rows
    HBK = nblk // 2
    assert HBK <= 31, f"int16 gather index overflow: HBK={HBK}"
    pad_local = (NC * 128) * HBK  # zeroed pad row, relative to half base
    # split each block's edges by src table-half; per-half tile counts are
    # cross-core maxes so all cores share the program structure
    half_lists = [[None] * (2 * nblk) for _ in range(NC)]
    for c in range(NC):
        s_c, d_c = core_edges[c]
        bounds = np.searchsorted(d_c, qbounds[c])
        cs = s_c // NPC
        ls = s_c % NPC
        bk = blk_of[cs, ls]
        hh = bk // HBK
        loc = (cs * 128 + pos_of[cs, ls]) * HBK + (bk % HBK)
        for b in range(nblk):
            e0, e1 = int(bounds[b]), int(bounds[b + 1])
            dp = d_c[e0:e1] - qbounds[c, b]
            for h in (0, 1):
                sel = hh[e0:e1] == h
                half_lists[c][b * 2 + h] = (
                    loc[e0:e1][sel].astype(np.int64),
                    dp[sel].astype(np.int64),
                )
    ntl = np.zeros(nblk, dtype=np.int64)
    nth = np.zeros(nblk, dtype=np.int64)
    for b in range(nblk):
        for c in range(NC):
            ntl[b] = max(ntl[b], -(-len(half_lists[c][b * 2][0]) // 128))
            nth[b] = max(nth[b], -(-len(half_lists[c][b * 2 + 1][0]) // 128))
    ntl[(ntl + nth) == 0] = 1  # every block needs >=1 tile (PSUM is read)
    tpb = ntl + nth
    tile_ofs = np.concatenate([[0], np.cumsum(tpb)])
    tot = int(tile_ofs[-1])
    SLOT_TOT = tot * 8
    slot_of = np.zeros((nblk, 2), dtype=np.int64)
    s = 0
    for b in range(nblk):
        slot_of[b, 0] = s
        s += int(ntl[b]) * 8
        slot_of[b, 1] = s
        s += int(nth[b]) * 8
    idxs = np.zeros((NC, 128, SLOT_TOT), dtype=np.int16)
    dstrel = np.full((NC, 128, tot), 255.0, dtype=np.float16)
    qq = np.arange(128)
    for c in range(NC):
        for b in range(nblk):
            t0 = int(tile_ofs[b])
            for h, nt in ((0, int(ntl[b])), (1, int(nth[b]))):
                if nt == 0:
                    continue
                loc, dp = half_lists[c][b * 2 + h]
                m = nt * 128
                L = np.full(m, pad_local, dtype=np.int64)
                L[: len(loc)] = loc
                s0 = int(slot_of[b, h])
                wr = L.reshape(m // 16, 16).T
                idxs[c, :, s0 : s0 + m // 16] = wr[qq % 16, :]
                dpp = np.full(m, 255, dtype=np.int64)
                dpp[: len(dp)] = dp
                dstrel[c, :, t0 : t0 + nt] = (
                    dpp.reshape(nt, 128).T.astype(np.float16)
                )
                t0 += nt
    return (
        nblk,
        ntl.astype(int).tolist(),
        nth.astype(int).tolist(),
        tile_ofs.astype(int).tolist(),
        tot,
        SLOT_TOT,
        slot_of,
        idxs,
        dstrel,
        qbounds,
    )


def kernel(
    x,
    edge_index,
    enc_W,
    enc_b,
    conv_t,
    conv_W1,
    conv_b1,
    conv_lng,
    conv_lnb,
    conv_W2,
    conv_b2,
    block_lng,
    block_lnb,
    fin_t,
    fin_W1,
    fin_b1,
    fin_bng,
    fin_bnb,
    fin_W2,
    fin_b2,
    _trace=False,
):
    global LAST_EXEC_NS
    _install_bir_patch()
    if _trace:
        _install_trace_hook()

    import concourse.bass as bass
    import concourse.mybir as mybir
    import concourse.tile as tile
    from concourse.bass import IndirectOffsetOnAxis
    from concourse.bass_utils import run_bass_kernel_spmd
    from concourse.masks import make_identity

    f32 = mybir.dt.float32
    f16 = mybir.dt.float16
    i32 = mybir.dt.int32
    AF = mybir.ActivationFunctionType
    OP = mybir.AluOpType
    AX = mybir.AxisListType

    from concourse import library_config
    from concourse.library_overlay import lower_extended_insts

    x = np.ascontiguousarray(np.asarray(x, dtype=np.float32))
    (NB, ntl, nth, tile_ofs, TOT, SLOT_TOT, slot_of, idxs_np, dstrel, qbounds) = (
        _preprocess_edges(np.asarray(edge_index))
    )
    NROWS_TAB = NC * 128
    ROWS_HALF = (NROWS_TAB + 8) * (NB // 2)
    i16 = mybir.dt.int16

    # ---------------- host-side parameter prep (replicated) ----------------
    rep = lambda v, w: np.ascontiguousarray(
        np.broadcast_to(np.asarray(v, np.float32).reshape(1, w), (128, w))
    )
    w1all = np.ascontiguousarray(
        np.concatenate(
            [np.asarray(conv_W1, np.float32), np.asarray(fin_W1, np.float32)[None]], 0
        )
    )  # [5, 64, 128]
    w2all = np.ascontiguousarray(
        np.concatenate(
            [np.asarray(conv_W2, np.float32), np.asarray(fin_W2, np.float32)[None]], 0
        )
    )  # [5, 128, 64]
    b1rep = np.concatenate(
        [rep(v, H2) for v in list(np.asarray(conv_b1, np.float32)) + [np.asarray(fin_b1)]],
        axis=1,
    )  # [128, 5*128]
    g_fin = np.asarray(fin_bng, np.float32) / np.sqrt(np.float32(1.0 + BN_EPS))
    garr = np.concatenate(
        [rep(v, H2) for v in list(np.asarray(conv_lng, np.float32)) + [g_fin]], axis=1
    )
    barr = np.concatenate(
        [rep(v, H2) for v in list(np.asarray(conv_lnb, np.float32)) + [np.asarray(fin_bnb)]],
        axis=1,
    )
    b2rep = np.concatenate(
        [rep(v, H) for v in list(np.asarray(conv_b2, np.float32)) + [np.asarray(fin_b2)]],
        axis=1,
    )  # [128, 5*64]
    blg = np.asarray(block_lng, np.float32)
    blb = np.asarray(block_lnb, np.float32)
    blkg = np.concatenate([rep(blg[i], H) for i in (1, 2, 3, 0)], axis=1)  # [128, 4*64]
    blkb = np.concatenate([rep(blb[i], H) for i in (1, 2, 3, 0)], axis=1)
    tvals = np.array(
        list(np.asarray(conv_t, np.float32)) + [float(np.asarray(fin_t))], np.float32
    )  # [5]
    tsc = np.ascontiguousarray(np.broadcast_to(tvals.reshape(1, 5), (128, 5)))
    tbi = np.ascontiguousarray(tsc * np.float32(GEN_EPS))
    iota128 = np.ascontiguousarray(
        np.broadcast_to(np.arange(128, dtype=np.float16).reshape(1, 128), (128, 128))
    )
    encW = np.ascontiguousarray(np.asarray(enc_W, np.float32))  # [128, 64]
    encb = rep(enc_b, H)

    # per-core transposed x, packed by (variable-width) blocks
    xT = np.zeros((NC, 128, NB * 128), dtype=np.float32)
    for c in range(NC):
        for b in range(NB):
            q0, q1 = int(qbounds[c, b]), int(qbounds[c, b + 1])
            if q1 > q0:
                xT[c, :, b * 128 : b * 128 + (q1 - q0)] = x[
                    c * NPC + q0 : c * NPC + q1
                ].T

    # ---------------- build the Bass program ----------------
    nc = bass.Bass(dynamic_dma_scratch_size=32768, num_swdge_queues=4)

    d_xT = nc.dram_tensor("xT", [128, NB * 128], f32, kind="ExternalInput")
    d_idx = nc.dram_tensor("idxs", [128, SLOT_TOT], i16, kind="ExternalInput")
    d_drl = nc.dram_tensor("dstrel", [128, TOT], f16, kind="ExternalInput")
    d_w1 = nc.dram_tensor("w1all", [5, H, H2], f32, kind="ExternalInput")
    d_w2 = nc.dram_tensor("w2all", [5, H2, H], f32, kind="ExternalInput")
    d_b1r = nc.dram_tensor("b1rep", [128, 5 * H2], f32, kind="ExternalInput")
    d_gar = nc.dram_tensor("garr", [128, 5 * H2], f32, kind="ExternalInput")
    d_bar = nc.dram_tensor("barr", [128, 5 * H2], f32, kind="ExternalInput")
    d_b2r = nc.dram_tensor("b2rep", [128, 5 * H], f32, kind="ExternalInput")
    d_blkg = nc.dram_tensor("blkg", [128, 4 * H], f32, kind="ExternalInput")
    d_blkb = nc.dram_tensor("blkb", [128, 4 * H], f32, kind="ExternalInput")
    d_tsc = nc.dram_tensor("tsc", [128, 5], f32, kind="ExternalInput")
    d_lneps = nc.dram_tensor("lneps", [128, 1], f32, kind="ExternalInput")
    d_tbi = nc.dram_tensor("tbi", [128, 5], f32, kind="ExternalInput")
    d_iota = nc.dram_tensor("iota", [128, 128], f16, kind="ExternalInput")
    d_encW = nc.dram_tensor("encW", [128, H], f32, kind="ExternalInput")
    d_encb = nc.dram_tensor("encb", [128, H], f32, kind="ExternalInput")
    d_out = nc.dram_tensor("out", [NB * 128, H], f32, kind="ExternalOutput")

    HB = NB // 2  # blocks per half (NB is even)
    C1 = HB * 128  # columns per half
    R2 = NROWS_TAB + 8  # rows per half incl zero-pad rows
    d_Tin_a = nc.dram_tensor("T_in_a", [128, C1], f16)
    d_Tin_b = nc.dram_tensor("T_in_b", [128, C1], f16)
    d_Ttab0 = nc.dram_tensor("T_tab0", [2 * R2, C1], f16, addr_space="Shared")
    d_Ttab1 = nc.dram_tensor("T_tab1", [2 * R2, C1], f16, addr_space="Shared")
    d_tabs = [d_Ttab0, d_Ttab1]
    t_half = []
    for d in (d_Ttab0[:], d_Ttab1[:]):
        full = d.rearrange("r (b f) -> (r b) f", f=H2)
        t_half.append([full[0:ROWS_HALF, :], full[ROWS_HALF : 2 * ROWS_HALF, :]])

    NBH = NB * H  # 3136

    with tile.TileContext(nc) as tc:
        with (
            tc.tile_pool(name="state", bufs=1) as st,
            tc.tile_pool(name="work", bufs=4) as wk,
            tc.tile_pool(name="big", bufs=1) as bg,
            tc.tile_pool(name="wload", bufs=2) as wl,
            tc.tile_pool(name="gat", bufs=3) as gp,
            tc.tile_pool(name="ohp", bufs=24) as ohp,
            tc.tile_pool(name="psum", bufs=4, space="PSUM") as pp,
            tc.tile_pool(name="psum1", bufs=1, space="PSUM") as pq,
        ):
            # persistent state / constants
            idx_sb = st.tile([128, SLOT_TOT], i16, tag="idx")
            nc.sync.dma_start(out=idx_sb[:], in_=d_idx[:])
            drl_sb = st.tile([128, TOT], f16, tag="drl")
            nc.sync.dma_start(out=drl_sb[:], in_=d_drl[:])
            iota_sb = st.tile([128, 128], f16, tag="iota")
            nc.sync.dma_start(out=iota_sb[:], in_=d_iota[:])
            ident = st.tile([128, 128], f32, tag="ident")
            make_identity(nc, ident[:])
            b1r_sb = st.tile([128, 5 * H2], f32, tag="b1r")
            nc.sync.dma_start(out=b1r_sb[:], in_=d_b1r[:])
            gar_sb = st.tile([128, 5 * H2], f32, tag="gar")
            nc.sync.dma_start(out=gar_sb[:], in_=d_gar[:])
            bar_sb = st.tile([128, 5 * H2], f32, tag="bar")
            nc.sync.dma_start(out=bar_sb[:], in_=d_bar[:])
            b2r_sb = st.tile([128, 5 * H], f32, tag="b2r")
            nc.sync.dma_start(out=b2r_sb[:], in_=d_b2r[:])
            blkg_sb = st.tile([128, 4 * H], f32, tag="blkg")
            nc.sync.dma_start(out=blkg_sb[:], in_=d_blkg[:])
            blkb_sb = st.tile([128, 4 * H], f32, tag="blkb")
            nc.sync.dma_start(out=blkb_sb[:], in_=d_blkb[:])
            tsc_sb = st.tile([128, 5], f32, tag="tsc")
            nc.sync.dma_start(out=tsc_sb[:], in_=d_tsc[:])
            lneps_sb = st.tile([128, 1], f32, tag="lneps")
            nc.sync.dma_start(out=lneps_sb[:], in_=d_lneps[:])
            tbi_sb = st.tile([128, 5], f32, tag="tbi")
            nc.sync.dma_start(out=tbi_sb[:], in_=d_tbi[:])
            encb_sb = st.tile([128, H], f32, tag="encb")
            nc.sync.dma_start(out=encb_sb[:], in_=d_encb[:])
            encW_sb = st.tile([128, H], f32, tag="encW")
            nc.sync.dma_start(out=encW_sb[:], in_=d_encW[:])
            xT_sb = bg.tile([128, NB * 128], f32, tag="bigA")
            nc.sync.dma_start(out=xT_sb[:], in_=d_xT[:])
            tloc_a = st.tile([128, C1], f16, tag="tloca")
            tloc_b = st.tile([128, C1], f16, tag="tlocb")
            tloc_init = tloc_b


            # zero the pad rows of both halves of both T tables (once)
            nc.gpsimd.memset(tloc_init[:], 0)
            for _tab in d_tabs:
                for _h in range(2):
                    nc.sync.dma_start(
                        out=_tab[_h * R2 + NROWS_TAB : (_h + 1) * R2, :],
                        in_=tloc_init[0:8, :],
                    )

            nc.gpsimd.load_library(library_config.mlp)
            _nt_regs = {}
            for _nt in range(1, 9):
                _nt_regs[_nt] = nc.gpsimd.to_reg(_nt * 128)
            _gq = [0]

            h_a = st.tile([128, NBH], f32, tag="h_a")
            h_b = st.tile([128, NBH], f32, tag="h_b")
            r_sb = st.tile([128, NBH], f32, tag="r_sb")

            # ---------------- encoder: h0 = x @ enc_W + enc_b -> r_sb ------
            for b in range(NB):
                ps_e = pq.tile([128, H], f32, tag="ph2")
                nc.tensor.matmul(
                    out=ps_e[:],
                    lhsT=xT_sb[:, b * 128 : (b + 1) * 128],
                    rhs=encW_sb[:],
                    start=True,
                    stop=True,
                )
                nc.vector.tensor_tensor(
                    out=r_sb[:, b * H : (b + 1) * H],
                    in0=ps_e[:],
                    in1=encb_sb[:],
                    op=OP.add,
                )

            h_cur, h_nxt = h_a, h_b

            def layer_norm_64(dst_ap, src_ap, g_ap, b_ap, relu):
                """dst = [relu](LN(src) * g + b) over 64 feats, node-major."""
                s1 = wk.tile([128, 1], f32, tag="s1")
                nc.vector.reduce_sum(out=s1[:], in_=src_ap, axis=AX.X)
                mu = wk.tile([128, 1], f32, tag="mu")
                nc.vector.tensor_scalar_mul(out=mu[:], in0=s1[:], scalar1=1.0 / H)
                hc = wk.tile([128, H], f32, tag="hc64")
                nc.vector.tensor_scalar_sub(out=hc[:], in0=src_ap, scalar1=mu[:])
                sq = wk.tile([128, H], f32, tag="sq64")
                nc.scalar.square(out=sq[:], in_=hc[:])
                s2 = wk.tile([128, 1], f32, tag="s2")
                nc.vector.reduce_sum(out=s2[:], in_=sq[:], axis=AX.X)
                sd = wk.tile([128, 1], f32, tag="sd")
                nc.scalar.activation(
                    out=sd[:], in_=s2[:], func=AF.Sqrt, bias=lneps_sb[:], scale=1.0 / H
                )
                rstd = wk.tile([128, 1], f32, tag="rstd")
                nc.vector.reciprocal(out=rstd[:], in_=sd[:])
                hn = wk.tile([128, H], f32, tag="hn64")
                nc.vector.tensor_scalar_mul(out=hn[:], in0=hc[:], scalar1=rstd[:])
                hg = wk.tile([128, H], f32, tag="hg64")
                nc.vector.tensor_tensor(out=hg[:], in0=hn[:], in1=g_ap, op=OP.mult)
                if relu:
                    hb_ = wk.tile([128, H], f32, tag="hb64")
                    nc.vector.tensor_tensor(out=hb_[:], in0=hg[:], in1=b_ap, op=OP.add)
                    nc.vector.tensor_scalar_max(out=dst_ap, in0=hb_[:], scalar1=0.0)
                else:
                    nc.vector.tensor_tensor(out=dst_ap, in0=hg[:], in1=b_ap, op=OP.add)

            def t_chunk(b, lidx):
                """tloc[:, b*128:(b+1)*128] = [P2|P1] of r_sb block b, layer lidx."""
                sl = slice(b * H, (b + 1) * H)
                tm = wk.tile([128, H], f32, tag="tm")
                nc.vector.tensor_scalar_max(out=tm[:], in0=r_sb[:, sl], scalar1=0.0)
                tp1 = wk.tile([128, H], f32, tag="tp1")
                nc.scalar.activation(
                    out=tp1[:],
                    in_=tm[:],
                    func=AF.Exp,
                    bias=tbi_sb[:, lidx : lidx + 1],
                    scale=tsc_sb[:, lidx : lidx + 1],
                )
                tme = wk.tile([128, H], f32, tag="tme")
                nc.vector.tensor_scalar_add(out=tme[:], in0=tm[:], scalar1=GEN_EPS)
                tp2 = wk.tile([128, H], f32, tag="tp2")
                nc.vector.tensor_tensor(
                    out=tp2[:], in0=tp1[:], in1=tme[:], op=OP.mult
                )
                if b < HB:
                    _tl, _off = tloc_a, b * H2
                else:
                    _tl, _off = tloc_b, (b - HB) * H2
                nc.vector.tensor_copy(out=_tl[:, _off : _off + H], in_=tp2[:])
                nc.vector.tensor_copy(out=_tl[:, _off + H : _off + H2], in_=tp1[:])

            def push_half1(tab):
                nc.sync.dma_start(out=d_Tin_a[:], in_=tloc_a[:])
                nc.gpsimd.collective_compute(
                    "AllGather",
                    OP.bypass,
                    replica_groups=[list(range(NC))],
                    ins=[d_Tin_a[:]],
                    outs=[tab[0:NROWS_TAB, :]],
                )

            def push_half2(tab):
                nc.sync.dma_start(out=d_Tin_b[:], in_=tloc_b[:])
                nc.gpsimd.collective_compute(
                    "AllGather",
                    OP.bypass,
                    replica_groups=[list(range(NC))],
                    ins=[d_Tin_b[:]],
                    outs=[tab[R2 : R2 + NROWS_TAB, :]],
                )

            for b in range(NB):
                t_chunk(b, 0)
                if b == HB - 1:
                    push_half1(d_tabs[0])
            push_half2(d_tabs[0])

            for l in range(5):
                w1_sb = wl.tile([H, H2], f32, tag="w1")
                nc.sync.dma_start(out=w1_sb[:], in_=d_w1[l])
                w2_sb = wl.tile([H2, H], f32, tag="w2")
                nc.sync.dma_start(out=w2_sb[:], in_=d_w2[l])

                # -------- edge + node phase per dst block ------------------
                for b in range(NB):
                    nl, nh = int(ntl[b]), int(nth[b])
                    nt_all = nl + nh
                    t0 = tile_ofs[b]
                    ps_agg = pp.tile([128, H2], f32, tag="pagg")
                    gts = []
                    for hh, nt in ((0, nl), (1, nh)):
                        if nt == 0:
                            continue
                        g_t = gp.tile([128, 16 * H2], f16, tag=f"g{hh}")
                        s0 = int(slot_of[b, hh])
                        # dma_gather breaks above 1024 idxs/instruction:
                        # split into <=8-tile chunks, round-robin SWDGE queues
                        for c0 in range(0, nt, 8):
                            cn = min(8, nt - c0)
                            g3v = g_t[
                                :, c0 * H2 : (c0 + cn) * H2
                            ].rearrange("p (t f) -> p t f", f=H2)
                            nc.gpsimd.dma_gather(
                                g3v,
                                t_half[l % 2][hh],
                                idx_sb[:, s0 + c0 * 8 : s0 + (c0 + cn) * 8],
                                cn * 128,
                                _nt_regs[cn],
                                H2,
                                queue_num=_gq[0] % 4,
                            )
                            _gq[0] += 1
                        gts.append((g_t, nt))
                    ti = 0
                    for g_t, nt in gts:
                        for t in range(nt):
                            col = t0 + ti
                            oh = ohp.tile([128, 128], f16, tag="oh")
                            nc.vector.tensor_tensor(
                                out=oh[:],
                                in0=iota_sb[:],
                                in1=drl_sb[:, col : col + 1].to_broadcast([128, 128]),
                                op=OP.is_equal,
                            )
                            nc.tensor.matmul(
                                out=ps_agg[:],
                                lhsT=oh[:],
                                rhs=g_t[:, t * H2 : (t + 1) * H2],
                                start=(ti == 0),
                                stop=(ti == nt_all - 1),
                            )
                            ti += 1
                    den = wk.tile([128, H], f32, tag="den")
                    nc.vector.tensor_scalar_max(
                        out=den[:], in0=ps_agg[:, H:H2], scalar1=1e-16
                    )
                    rec = wk.tile([128, H], f32, tag="rec")
                    nc.vector.reciprocal(out=rec[:], in_=den[:])
                    agg = wk.tile([128, H], f32, tag="agg")
                    nc.vector.tensor_tensor(
                        out=agg[:], in0=ps_agg[:, 0:H], in1=rec[:], op=OP.mult
                    )
                    # ---- MLP ----
                    sl = slice(b * H, (b + 1) * H)
                    u = wk.tile([128, H], f32, tag="u")
                    nc.vector.tensor_tensor(
                        out=u[:], in0=agg[:], in1=r_sb[:, sl], op=OP.add
                    )
                    ps_t = pq.tile([H, 128], f32, tag="ptr")
                    nc.tensor.transpose(out=ps_t[:], in_=u[:], identity=ident[:])
                    uT = wk.tile([H, 128], f32, tag="uT")
                    nc.scalar.copy(out=uT[:], in_=ps_t[:])
                    ps_h1 = pq.tile([128, H2], f32, tag="ph1")
                    nc.tensor.matmul(
                        out=ps_h1[:], lhsT=uT[:], rhs=w1_sb[:], start=True, stop=True
                    )
                    l2 = slice(l * H2, (l + 1) * H2)
                    h1 = wk.tile([128, H2], f32, tag="h1")
                    nc.vector.tensor_tensor(
                        out=h1[:], in0=ps_h1[:], in1=b1r_sb[:, l2], op=OP.add
                    )
                    if l < 4:
                        # conv LayerNorm over 128 feats
                        s1 = wk.tile([128, 1], f32, tag="cs1")
                        nc.vector.reduce_sum(out=s1[:], in_=h1[:], axis=AX.X)
                        mu = wk.tile([128, 1], f32, tag="cmu")
                        nc.vector.tensor_scalar_mul(
                            out=mu[:], in0=s1[:], scalar1=1.0 / H2
                        )
                        hc = wk.tile([128, H2], f32, tag="chc")
                        nc.vector.tensor_scalar_sub(out=hc[:], in0=h1[:], scalar1=mu[:])
                        sq = wk.tile([128, H2], f32, tag="csq")
                        nc.scalar.square(out=sq[:], in_=hc[:])
                        s2 = wk.tile([128, 1], f32, tag="cs2")
                        nc.vector.reduce_sum(out=s2[:], in_=sq[:], axis=AX.X)
                        sd = wk.tile([128, 1], f32, tag="csd")
                        nc.scalar.activation(
                            out=sd[:],
                            in_=s2[:],
                            func=AF.Sqrt,
                            bias=lneps_sb[:],
                            scale=1.0 / H2,
                        )
                        rstd = wk.tile([128, 1], f32, tag="crstd")
                        nc.vector.reciprocal(out=rstd[:], in_=sd[:])
                        hn = wk.tile([128, H2], f32, tag="chn")
                        nc.vector.tensor_scalar_mul(
                            out=hn[:], in0=hc[:], scalar1=rstd[:]
                        )
                    else:
                        hn = h1
                    hg = wk.tile([128, H2], f32, tag="chg")
                    nc.vector.tensor_tensor(
                        out=hg[:], in0=hn[:], in1=gar_sb[:, l2], op=OP.mult
                    )
                    hb2 = wk.tile([128, H2], f32, tag="chb")
                    nc.vector.tensor_tensor(
                        out=hb2[:], in0=hg[:], in1=bar_sb[:, l2], op=OP.add
                    )
                    r1 = wk.tile([128, H2], f32, tag="r1")
                    nc.vector.tensor_scalar_max(out=r1[:], in0=hb2[:], scalar1=0.0)
                    ps_t2 = pq.tile([128, 128], f32, tag="ptr2")
                    nc.tensor.transpose(out=ps_t2[:], in_=r1[:], identity=ident[:])
                    r1T = wk.tile([128, 128], f32, tag="r1T")
                    nc.scalar.copy(out=r1T[:], in_=ps_t2[:])
                    ps_h2 = pq.tile([128, H], f32, tag="ph2")
                    nc.tensor.matmul(
                        out=ps_h2[:], lhsT=r1T[:], rhs=w2_sb[:], start=True, stop=True
                    )
                    lh = slice(l * H, (l + 1) * H)
                    if l == 0:
                        nc.vector.tensor_tensor(
                            out=h_nxt[:, sl], in0=ps_h2[:], in1=b2r_sb[:, lh], op=OP.add
                        )
                    elif l < 4:
                        co = wk.tile([128, H], f32, tag="co")
                        nc.vector.tensor_tensor(
                            out=co[:], in0=ps_h2[:], in1=b2r_sb[:, lh], op=OP.add
                        )
                        nc.vector.tensor_tensor(
                            out=h_nxt[:, sl], in0=co[:], in1=h_cur[:, sl], op=OP.add
                        )
                    else:
                        co = wk.tile([128, H], f32, tag="co")
                        nc.vector.tensor_tensor(
                            out=co[:], in0=ps_h2[:], in1=b2r_sb[:, lh], op=OP.add
                        )
                        nc.sync.dma_start(
                            out=d_out[b * 128 : (b + 1) * 128, :], in_=co[:]
                        )
                    if l < 4:
                        # r for conv l+1 from h_nxt, then its T chunk
                        gsl = slice(l * H, (l + 1) * H)
                        layer_norm_64(
                            r_sb[:, sl],
                            h_nxt[:, sl],
                            blkg_sb[:, gsl],
                            blkb_sb[:, gsl],
                            relu=True,
                        )
                        t_chunk(b, l + 1)
                        if b == HB - 1:
                            push_half1(d_tabs[(l + 1) % 2])
                if l < 4:
                    push_half2(d_tabs[(l + 1) % 2])
                    h_cur, h_nxt = h_nxt, h_cur

    in_maps = []
    for c in range(NC):
        in_maps.append(
            {
                "xT": xT[c],
                "idxs": np.ascontiguousarray(idxs_np[c]),
                "dstrel": np.ascontiguousarray(dstrel[c]),
                "w1all": w1all,
                "w2all": w2all,
                "b1rep": b1rep,
                "garr": garr,
                "barr": barr,
                "b2rep": b2rep,
                "blkg": blkg,
                "blkb": blkb,
                "tsc": tsc,
                "lneps": np.full((128, 1), LN_EPS, np.float32),
                "tbi": tbi,
                "iota": iota128,
                "encW": encW,
                "encb": encb,
            }
        )
    lower_extended_insts(nc)
    res = run_bass_kernel_spmd(nc, in_maps, list(range(NC)), trace=_trace)
    LAST_EXEC_NS = res.exec_time_ns
    out = np.empty((N, H), dtype=np.float32)
    for c in range(NC):
        oc = res.results[c]["out"]
        for b in range(NB):
            q0, q1 = int(qbounds[c, b]), int(qbounds[c, b + 1])
            if q1 > q0:
                out[c * NPC + q0 : c * NPC + q1] = oc[b * 128 : b * 128 + (q1 - q0)]
    return out.astype(np.float32)



# revision 38
# speedup vs baseline: 1.7385x; 1.4341x over previous
"""DeeperGCN forward on 8 Trainium2 NeuronCores (Bass/Tile).

Strategy (graph/data parallel, dst-node sharding):
- Nodes sharded 6250/core. Edges assigned to the core owning their dst,
  sorted by dst, tiled 128/tile within 128-node dst blocks.
- Per conv layer, each core computes node tables P1 = exp(t*msg),
  P2 = P1*msg (msg = relu(r)+eps) for its own nodes; tables are
  AllGathered (fp16) so every core holds the full [N,128] T=[P2|P1].
- Edge phase: indirect-DMA gather of T rows by src, one-hot matmul
  aggregation (onehot[e,dst].T @ T_rows) accumulated in PSUM per dst
  block -> num/den -> agg = num/max(den,1e-16).  This reproduces the
  softmax aggregation exactly (max-subtraction cancels in num/den).
- Node phase: root residual + MLP (Linear->LN/BN->ReLU->Linear) with
  TensorE matmuls/transposes, DVE/ACT elementwise, node-major layout.
"""

import json
import sys
import types

import numpy as np

sys.path.insert(0, "/opt/trn_rl_repo")

# ---------------------------------------------------------------------------
# Workaround: this walrus build supports only ONE semaphore wait per
# instruction; Tile attaches several. Split extras onto NoOp instructions
# at BIR-JSON serialization time.
# ---------------------------------------------------------------------------
_PATCHED = False


def _install_bir_patch():
    global _PATCHED
    if _PATCHED:
        return
    _PATCHED = True
    import concourse.bass as bass

    orig = bass.Bass.to_json_bytes

    def patched(self):
        data = json.loads(orig(self).decode())
        ctr = 0
        for fn in data.get("functions", []):
            for bb in fn.get("blocks", []):
                new_insts = []
                for inst in bb.get("instructions", []):
                    si = inst.get("sync_info")
                    waits = (si or {}).get("on_wait") or []
                    if len(waits) > 1:
                        for w in waits[:-1]:
                            ctr += 1
                            nop = {
                                "engine": inst["engine"],
                                "ins": [],
                                "outs": [],
                                "name": f"{inst['name']}-sw{ctr}",
                                "opcode": "NoOp",
                                "sync_info": {"on_update": [], "on_wait": [w]},
                            }
                            if "debug" in inst:
                                nop["debug"] = inst["debug"]
                            new_insts.append(nop)
                        si["on_wait"] = [waits[-1]]
                    new_insts.append(inst)
                bb["instructions"] = new_insts
        return json.dumps(data).encode()

    bass.Bass.to_json_bytes = patched


def _install_trace_hook():
    """Optional: register the NTFF profiling hook (for test.py timing)."""
    import antenv

    if "antenv.axon_hooks" in sys.modules:
        return
    _m = types.ModuleType("antenv.axon_hooks")
    _m._hook = None
    _m.set_axon_ntff_profile_hook = lambda h: setattr(_m, "_hook", h)
    _m.get_axon_ntff_profile_hook = lambda: _m._hook
    sys.modules["antenv.axon_hooks"] = _m
    antenv.axon_hooks = _m
    try:
        from trn_agent_boot.trn_boot import _ntff_profile_via_ctypes

        _m._hook = _ntff_profile_via_ctypes("/opt/axon/libaxon_pjrt.so")
    except Exception:
        pass


N, NC, NPC = 50000, 8, 6250
H = 64
H2 = 128
F_IN = 128
LN_EPS = 1e-5
BN_EPS = 1e-5
GEN_EPS = 1e-7

LAST_EXEC_NS = None


def _preprocess_edges(edge_index):
    """Per-core dst-sorted edges, variable node-range blocks with a fixed
    2048-edge budget (16 tiles of 128) per block. Returns shared tile
    geometry + per-core index arrays + per-core block node bounds."""
    CAP = 2048
    TPB = CAP // 128  # 16 tiles per block
    src = np.asarray(edge_index[0], dtype=np.int64)
    dst = np.asarray(edge_index[1], dtype=np.int64)
    core_edges = []
    core_qb = []
    for c in range(NC):
        m = (dst >= c * NPC) & (dst < (c + 1) * NPC)
        s_c = src[m]
        d_c = dst[m] - c * NPC
        order = np.argsort(d_c, kind="stable")
        s_c, d_c = s_c[order], d_c[order]
        deg = np.bincount(d_c, minlength=NPC)
        qb = [0]
        nodes_in = 0
        cum = 0
        for n in range(NPC):
            dn = int(deg[n])
            if nodes_in == 128 or cum + dn > CAP:
                qb.append(n)
                nodes_in = 0
                cum = 0
            nodes_in += 1
            cum += dn
        qb.append(NPC)
        core_edges.append((s_c, d_c))
        core_qb.append(qb)
    nblk = max(len(qb) - 1 for qb in core_qb)
    nblk += nblk % 2  # even block count for half-split AllGather
    # pad bounds to nblk+1 entries (trailing empty blocks)
    qbounds = np.full((NC, nblk + 1), NPC, dtype=np.int64)
    for c in range(NC):
        qb = core_qb[c]
        qbounds[c, : len(qb)] = qb
    # node -> (block, pos) per core
    blk_of = np.zeros((NC, NPC), dtype=np.int64)
    pos_of = np.zeros((NC, NPC), dtype=np.int64)
    for c in range(NC):
        for b in range(nblk):
            q0, q1 = int(qbounds[c, b]), int(qbounds[c, b + 1])
            if q1 > q0:
                blk_of[c, q0:q1] = b
                pos_of[c, q0:q1] = np.arange(q1 - q0)
    R2 = NC * 128 + 8  # rows per half-table incl zero-pad rows
    HBK = nblk // 2
    assert HBK <= 31, f"int16 gather index overflow: HBK={HBK}"
    pad_local = (NC * 128) * HBK  # zeroed pad row, relative to half base
    # split each block's edges by src table-half; per-half tile counts are
    # cross-core maxes so all cores share the program structure
    half_lists = [[None] * (2 * nblk) for _ in range(NC)]
    for c in range(NC):
        s_c, d_c = core_edges[c]
        bounds = np.searchsorted(d_c, qbounds[c])
        cs = s_c // NPC
        ls = s_c % NPC
        bk = blk_of[cs, ls]
        hh = bk // HBK
        loc = (cs * 128 + pos_of[cs, ls]) * HBK + (bk % HBK)
        for b in range(nblk):
            e0, e1 = int(bounds[b]), int(bounds[b + 1])
            dp = d_c[e0:e1] - qbounds[c, b]
            for h in (0, 1):
                sel = hh[e0:e1] == h
                half_lists[c][b * 2 + h] = (
                    loc[e0:e1][sel].astype(np.int64),
                    dp[sel].astype(np.int64),
                )
    ntl = np.zeros(nblk, dtype=np.int64)
    nth = np.zeros(nblk, dtype=np.int64)
    for b in range(nblk):
        for c in range(NC):
            ntl[b] = max(ntl[b], -(-len(half_lists[c][b * 2][0]) // 128))
            nth[b] = max(nth[b], -(-len(half_lists[c][b * 2 + 1][0]) // 128))
    ntl[(ntl + nth) == 0] = 1  # every block needs >=1 tile (PSUM is read)
    tpb = ntl + nth
    tile_ofs = np.concatenate([[0], np.cumsum(tpb)])
    tot = int(tile_ofs[-1])
    SLOT_TOT = tot * 8
    slot_of = np.zeros((nblk, 2), dtype=np.int64)
    s = 0
    for b in range(nblk):
        slot_of[b, 0] = s
        s += int(ntl[b]) * 8
        slot_of[b, 1] = s
        s += int(nth[b]) * 8
    import ml_dtypes

    idxs = np.zeros((NC, 128, SLOT_TOT), dtype=np.int16)
    oh = np.zeros((NC, 128, tot * 128), dtype=ml_dtypes.float8_e4m3)
    qq = np.arange(128)
    for c in range(NC):
        for b in range(nblk):
            t0 = int(tile_ofs[b])
            for h, nt in ((0, int(ntl[b])), (1, int(nth[b]))):
                if nt == 0:
                    continue
                loc, dp = half_lists[c][b * 2 + h]
                m = nt * 128
                L = np.full(m, pad_local, dtype=np.int64)
                L[: len(loc)] = loc
                s0 = int(slot_of[b, h])
                wr = L.reshape(m // 16, 16).T
                idxs[c, :, s0 : s0 + m // 16] = wr[qq % 16, :]
                # fp8 one-hot tiles: edge j -> tile t0+j//128, lane j%128
                j = np.arange(len(dp))
                tt = t0 + j // 128
                oh[c, j % 128, tt * 128 + dp] = 1.0
                t0 += nt
    return (
        nblk,
        ntl.astype(int).tolist(),
        nth.astype(int).tolist(),
        tile_ofs.astype(int).tolist(),
        tot,
        SLOT_TOT,
        slot_of,
        idxs,
        oh,
        qbounds,
    )


def kernel(
    x,
    edge_index,
    enc_W,
    enc_b,
    conv_t,
    conv_W1,
    conv_b1,
    conv_lng,
    conv_lnb,
    conv_W2,
    conv_b2,
    block_lng,
    block_lnb,
    fin_t,
    fin_W1,
    fin_b1,
    fin_bng,
    fin_bnb,
    fin_W2,
    fin_b2,
    _trace=False,
):
    global LAST_EXEC_NS
    _install_bir_patch()
    if _trace:
        _install_trace_hook()

    import concourse.bass as bass
    import concourse.mybir as mybir
    import concourse.tile as tile
    from concourse.bass import IndirectOffsetOnAxis
    from concourse.bass_utils import run_bass_kernel_spmd
    from concourse.masks import make_identity

    f32 = mybir.dt.float32
    f16 = mybir.dt.float16
    f8 = mybir.dt.float8e4
    i32 = mybir.dt.int32
    AF = mybir.ActivationFunctionType
    OP = mybir.AluOpType
    AX = mybir.AxisListType

    from concourse import library_config
    from concourse.library_overlay import lower_extended_insts

    x = np.ascontiguousarray(np.asarray(x, dtype=np.float32))
    (NB, ntl, nth, tile_ofs, TOT, SLOT_TOT, slot_of, idxs_np, oh_np, qbounds) = (
        _preprocess_edges(np.asarray(edge_index))
    )
    NROWS_TAB = NC * 128
    ROWS_HALF = (NROWS_TAB + 8) * (NB // 2)
    i16 = mybir.dt.int16

    # ---------------- host-side parameter prep (replicated) ----------------
    rep = lambda v, w: np.ascontiguousarray(
        np.broadcast_to(np.asarray(v, np.float32).reshape(1, w), (128, w))
    )
    w1all = np.ascontiguousarray(
        np.concatenate(
            [np.asarray(conv_W1, np.float32), np.asarray(fin_W1, np.float32)[None]], 0
        )
    )  # [5, 64, 128]
    w2all = np.ascontiguousarray(
        np.concatenate(
            [np.asarray(conv_W2, np.float32), np.asarray(fin_W2, np.float32)[None]], 0
        )
    )  # [5, 128, 64]
    b1rep = np.concatenate(
        [rep(v, H2) for v in list(np.asarray(conv_b1, np.float32)) + [np.asarray(fin_b1)]],
        axis=1,
    )  # [128, 5*128]
    g_fin = np.asarray(fin_bng, np.float32) / np.sqrt(np.float32(1.0 + BN_EPS))
    garr = np.concatenate(
        [rep(v, H2) for v in list(np.asarray(conv_lng, np.float32)) + [g_fin]], axis=1
    )
    barr = np.concatenate(
        [rep(v, H2) for v in list(np.asarray(conv_lnb, np.float32)) + [np.asarray(fin_bnb)]],
        axis=1,
    )
    b2rep = np.concatenate(
        [rep(v, H) for v in list(np.asarray(conv_b2, np.float32)) + [np.asarray(fin_b2)]],
        axis=1,
    )  # [128, 5*64]
    blg = np.asarray(block_lng, np.float32)
    blb = np.asarray(block_lnb, np.float32)
    blkg = np.concatenate([rep(blg[i], H) for i in (1, 2, 3, 0)], axis=1)  # [128, 4*64]
    blkb = np.concatenate([rep(blb[i], H) for i in (1, 2, 3, 0)], axis=1)
    tvals = np.array(
        list(np.asarray(conv_t, np.float32)) + [float(np.asarray(fin_t))], np.float32
    )  # [5]
    tsc = np.ascontiguousarray(np.broadcast_to(tvals.reshape(1, 5), (128, 5)))
    tbi = np.ascontiguousarray(tsc * np.float32(GEN_EPS))
    encW = np.ascontiguousarray(np.asarray(enc_W, np.float32))  # [128, 64]
    encb = rep(enc_b, H)

    # per-core transposed x, packed by (variable-width) blocks
    xT = np.zeros((NC, 128, NB * 128), dtype=np.float32)
    for c in range(NC):
        for b in range(NB):
            q0, q1 = int(qbounds[c, b]), int(qbounds[c, b + 1])
            if q1 > q0:
                xT[c, :, b * 128 : b * 128 + (q1 - q0)] = x[
                    c * NPC + q0 : c * NPC + q1
                ].T

    # ---------------- build the Bass program ----------------
    nc = bass.Bass(dynamic_dma_scratch_size=32768, num_swdge_queues=4)

    d_xT = nc.dram_tensor("xT", [128, NB * 128], f32, kind="ExternalInput")
    d_idx = nc.dram_tensor("idxs", [128, SLOT_TOT], i16, kind="ExternalInput")
    d_oh = nc.dram_tensor("oh", [128, TOT * H2], mybir.dt.float8e4, kind="ExternalInput")
    d_w1 = nc.dram_tensor("w1all", [5, H, H2], f32, kind="ExternalInput")
    d_w2 = nc.dram_tensor("w2all", [5, H2, H], f32, kind="ExternalInput")
    d_b1r = nc.dram_tensor("b1rep", [128, 5 * H2], f32, kind="ExternalInput")
    d_gar = nc.dram_tensor("garr", [128, 5 * H2], f32, kind="ExternalInput")
    d_bar = nc.dram_tensor("barr", [128, 5 * H2], f32, kind="ExternalInput")
    d_b2r = nc.dram_tensor("b2rep", [128, 5 * H], f32, kind="ExternalInput")
    d_blkg = nc.dram_tensor("blkg", [128, 4 * H], f32, kind="ExternalInput")
    d_blkb = nc.dram_tensor("blkb", [128, 4 * H], f32, kind="ExternalInput")
    d_tsc = nc.dram_tensor("tsc", [128, 5], f32, kind="ExternalInput")
    d_lneps = nc.dram_tensor("lneps", [128, 1], f32, kind="ExternalInput")
    d_tbi = nc.dram_tensor("tbi", [128, 5], f32, kind="ExternalInput")
    d_encW = nc.dram_tensor("encW", [128, H], f32, kind="ExternalInput")
    d_encb = nc.dram_tensor("encb", [128, H], f32, kind="ExternalInput")
    d_out = nc.dram_tensor("out", [NB * 128, H], f32, kind="ExternalOutput")

    HB = NB // 2  # blocks per half (NB is even)
    C1 = HB * 128  # columns per half
    R2 = NROWS_TAB + 8  # rows per half incl zero-pad rows
    d_Tin_a = nc.dram_tensor("T_in_a", [128, C1], f16)
    d_Tin_b = nc.dram_tensor("T_in_b", [128, C1], f16)
    d_Ttab0 = nc.dram_tensor("T_tab0", [2 * R2, C1], f16, addr_space="Shared")
    d_Ttab1 = nc.dram_tensor("T_tab1", [2 * R2, C1], f16, addr_space="Shared")
    d_tabs = [d_Ttab0, d_Ttab1]
    t_half = []
    for d in (d_Ttab0[:], d_Ttab1[:]):
        full = d.rearrange("r (b f) -> (r b) f", f=H2)
        t_half.append([full[0:ROWS_HALF, :], full[ROWS_HALF : 2 * ROWS_HALF, :]])

    NBH = NB * H  # 3136

    with tile.TileContext(nc) as tc:
        with (
            tc.tile_pool(name="state", bufs=1) as st,
            tc.tile_pool(name="work", bufs=4) as wk,
            tc.tile_pool(name="big", bufs=1) as bg,
            tc.tile_pool(name="wload", bufs=2) as wl,
            tc.tile_pool(name="gat", bufs=3) as gp,
            tc.tile_pool(name="ohp", bufs=3) as ohp,
            tc.tile_pool(name="psum", bufs=4, space="PSUM") as pp,
            tc.tile_pool(name="psum1", bufs=1, space="PSUM") as pq,
        ):
            # persistent state / constants
            idx_sb = st.tile([128, SLOT_TOT], i16, tag="idx")
            nc.sync.dma_start(out=idx_sb[:], in_=d_idx[:])
            ident = st.tile([128, 128], f32, tag="ident")
            make_identity(nc, ident[:])
            b1r_sb = st.tile([128, 5 * H2], f32, tag="b1r")
            nc.sync.dma_start(out=b1r_sb[:], in_=d_b1r[:])
            gar_sb = st.tile([128, 5 * H2], f32, tag="gar")
            nc.sync.dma_start(out=gar_sb[:], in_=d_gar[:])
            bar_sb = st.tile([128, 5 * H2], f32, tag="bar")
            nc.sync.dma_start(out=bar_sb[:], in_=d_bar[:])
            b2r_sb = st.tile([128, 5 * H], f32, tag="b2r")
            nc.sync.dma_start(out=b2r_sb[:], in_=d_b2r[:])
            blkg_sb = st.tile([128, 4 * H], f32, tag="blkg")
            nc.sync.dma_start(out=blkg_sb[:], in_=d_blkg[:])
            blkb_sb = st.tile([128, 4 * H], f32, tag="blkb")
            nc.sync.dma_start(out=blkb_sb[:], in_=d_blkb[:])
            tsc_sb = st.tile([128, 5], f32, tag="tsc")
            nc.sync.dma_start(out=tsc_sb[:], in_=d_tsc[:])
            lneps_sb = st.tile([128, 1], f32, tag="lneps")
            nc.sync.dma_start(out=lneps_sb[:], in_=d_lneps[:])
            tbi_sb = st.tile([128, 5], f32, tag="tbi")
            nc.sync.dma_start(out=tbi_sb[:], in_=d_tbi[:])
            encb_sb = st.tile([128, H], f32, tag="encb")
            nc.sync.dma_start(out=encb_sb[:], in_=d_encb[:])
            encW_sb = st.tile([128, H], f32, tag="encW")
            nc.sync.dma_start(out=encW_sb[:], in_=d_encW[:])
            xT_sb = bg.tile([128, NB * 128], f32, tag="bigA")
            nc.sync.dma_start(out=xT_sb[:], in_=d_xT[:])
            tloc_a = st.tile([128, C1], f16, tag="tloca")
            tloc_b = st.tile([128, C1], f16, tag="tlocb")
            tloc_init = tloc_b


            # zero the pad rows of both halves of both T tables (once)
            nc.gpsimd.memset(tloc_init[:], 0)
            for _tab in d_tabs:
                for _h in range(2):
                    nc.sync.dma_start(
                        out=_tab[_h * R2 + NROWS_TAB : (_h + 1) * R2, :],
                        in_=tloc_init[0:8, :],
                    )

            nc.gpsimd.load_library(library_config.mlp)
            _nt_regs = {}
            for _nt in range(1, 9):
                _nt_regs[_nt] = nc.gpsimd.to_reg(_nt * 128)
            _gq = [0]

            h_a = st.tile([128, NBH], f32, tag="h_a")
            h_b = st.tile([128, NBH], f32, tag="h_b")
            r_sb = st.tile([128, NBH], f32, tag="r_sb")

            # ---------------- encoder: h0 = x @ enc_W + enc_b -> r_sb ------
            for b in range(NB):
                ps_e = pq.tile([128, H], f32, tag="ph2")
                nc.tensor.matmul(
                    out=ps_e[:],
                    lhsT=xT_sb[:, b * 128 : (b + 1) * 128],
                    rhs=encW_sb[:],
                    start=True,
                    stop=True,
                )
                nc.vector.tensor_tensor(
                    out=r_sb[:, b * H : (b + 1) * H],
                    in0=ps_e[:],
                    in1=encb_sb[:],
                    op=OP.add,
                )

            h_cur, h_nxt = h_a, h_b

            def layer_norm_64(dst_ap, src_ap, g_ap, b_ap, relu):
                """dst = [relu](LN(src) * g + b) over 64 feats, node-major."""
                s1 = wk.tile([128, 1], f32, tag="s1")
                nc.vector.reduce_sum(out=s1[:], in_=src_ap, axis=AX.X)
                mu = wk.tile([128, 1], f32, tag="mu")
                nc.vector.tensor_scalar_mul(out=mu[:], in0=s1[:], scalar1=1.0 / H)
                hc = wk.tile([128, H], f32, tag="hc64")
                nc.vector.tensor_scalar_sub(out=hc[:], in0=src_ap, scalar1=mu[:])
                sq = wk.tile([128, H], f32, tag="sq64")
                nc.scalar.square(out=sq[:], in_=hc[:])
                s2 = wk.tile([128, 1], f32, tag="s2")
                nc.vector.reduce_sum(out=s2[:], in_=sq[:], axis=AX.X)
                sd = wk.tile([128, 1], f32, tag="sd")
                nc.scalar.activation(
                    out=sd[:], in_=s2[:], func=AF.Sqrt, bias=lneps_sb[:], scale=1.0 / H
                )
                rstd = wk.tile([128, 1], f32, tag="rstd")
                nc.vector.reciprocal(out=rstd[:], in_=sd[:])
                hn = wk.tile([128, H], f32, tag="hn64")
                nc.vector.tensor_scalar_mul(out=hn[:], in0=hc[:], scalar1=rstd[:])
                hg = wk.tile([128, H], f32, tag="hg64")
                nc.vector.tensor_tensor(out=hg[:], in0=hn[:], in1=g_ap, op=OP.mult)
                if relu:
                    hb_ = wk.tile([128, H], f32, tag="hb64")
                    nc.vector.tensor_tensor(out=hb_[:], in0=hg[:], in1=b_ap, op=OP.add)
                    nc.vector.tensor_scalar_max(out=dst_ap, in0=hb_[:], scalar1=0.0)
                else:
                    nc.vector.tensor_tensor(out=dst_ap, in0=hg[:], in1=b_ap, op=OP.add)

            def t_chunk(b, lidx, relu=False):
                """tloc[:, b*128:(b+1)*128] = [P2|P1] of r_sb block b, layer lidx.
                relu only needed for layer 0 (encoder output isn't relu'd)."""
                sl = slice(b * H, (b + 1) * H)
                if relu:
                    tm = wk.tile([128, H], f32, tag="tm")
                    nc.vector.tensor_scalar_max(
                        out=tm[:], in0=r_sb[:, sl], scalar1=0.0
                    )
                    tm_ap = tm[:]
                else:
                    tm_ap = r_sb[:, sl]
                tp1 = wk.tile([128, H], f32, tag="tp1")
                nc.scalar.activation(
                    out=tp1[:],
                    in_=tm_ap,
                    func=AF.Exp,
                    bias=tbi_sb[:, lidx : lidx + 1],
                    scale=tsc_sb[:, lidx : lidx + 1],
                )
                tme = wk.tile([128, H], f32, tag="tme")
                nc.vector.tensor_scalar_add(out=tme[:], in0=tm_ap, scalar1=GEN_EPS)
                tp2 = wk.tile([128, H], f32, tag="tp2")
                nc.vector.tensor_tensor(
                    out=tp2[:], in0=tp1[:], in1=tme[:], op=OP.mult
                )
                if b < HB:
                    _tl, _off = tloc_a, b * H2
                else:
                    _tl, _off = tloc_b, (b - HB) * H2
                nc.vector.tensor_copy(out=_tl[:, _off : _off + H], in_=tp2[:])
                nc.vector.tensor_copy(out=_tl[:, _off + H : _off + H2], in_=tp1[:])

            def push_half1(tab):
                nc.sync.dma_start(out=d_Tin_a[:], in_=tloc_a[:])
                nc.gpsimd.collective_compute(
                    "AllGather",
                    OP.bypass,
                    replica_groups=[list(range(NC))],
                    ins=[d_Tin_a[:]],
                    outs=[tab[0:NROWS_TAB, :]],
                )

            def push_half2(tab):
                nc.sync.dma_start(out=d_Tin_b[:], in_=tloc_b[:])
                nc.gpsimd.collective_compute(
                    "AllGather",
                    OP.bypass,
                    replica_groups=[list(range(NC))],
                    ins=[d_Tin_b[:]],
                    outs=[tab[R2 : R2 + NROWS_TAB, :]],
                )

            for b in range(NB):
                t_chunk(b, 0, relu=True)
                if b == HB - 1:
                    push_half1(d_tabs[0])
            push_half2(d_tabs[0])

            for l in range(5):
                w1_sb = wl.tile([H, H2], f32, tag="w1")
                nc.sync.dma_start(out=w1_sb[:], in_=d_w1[l])
                w2_sb = wl.tile([H2, H], f32, tag="w2")
                nc.sync.dma_start(out=w2_sb[:], in_=d_w2[l])

                # -------- edge + node phase per dst block ------------------
                for b in range(NB):
                    nl, nh = int(ntl[b]), int(nth[b])
                    nt_all = nl + nh
                    t0 = tile_ofs[b]
                    ps_agg = pp.tile([128, H2], f32, tag="pagg")
                    gts = []
                    for hh, nt in ((0, nl), (1, nh)):
                        if nt == 0:
                            continue
                        g_t = gp.tile([128, 16 * H2], f16, tag=f"g{hh}")
                        s0 = int(slot_of[b, hh])
                        # dma_gather breaks above 1024 idxs/instruction:
                        # split into <=8-tile chunks, round-robin SWDGE queues
                        for c0 in range(0, nt, 8):
                            cn = min(8, nt - c0)
                            g3v = g_t[
                                :, c0 * H2 : (c0 + cn) * H2
                            ].rearrange("p (t f) -> p t f", f=H2)
                            nc.gpsimd.dma_gather(
                                g3v,
                                t_half[l % 2][hh],
                                idx_sb[:, s0 + c0 * 8 : s0 + (c0 + cn) * 8],
                                cn * 128,
                                _nt_regs[cn],
                                H2,
                                queue_num=_gq[0] % 4,
                            )
                            _gq[0] += 1
                        gts.append((g_t, nt))
                    oh_t = ohp.tile([128, 18 * H2], f8, tag="oh")
                    nc.sync.dma_start(
                        out=oh_t[:, : nt_all * H2],
                        in_=d_oh[:, t0 * H2 : (t0 + nt_all) * H2],
                    )
                    ti = 0
                    for g_t, nt in gts:
                        for t in range(nt):
                            nc.tensor.matmul(
                                out=ps_agg[:],
                                lhsT=oh_t[:, ti * H2 : (ti + 1) * H2],
                                rhs=g_t[:, t * H2 : (t + 1) * H2],
                                start=(ti == 0),
                                stop=(ti == nt_all - 1),
                            )
                            ti += 1
                    den = wk.tile([128, H], f32, tag="den")
                    nc.vector.tensor_scalar_max(
                        out=den[:], in0=ps_agg[:, H:H2], scalar1=1e-16
                    )
                    rec = wk.tile([128, H], f32, tag="rec")
                    nc.vector.reciprocal(out=rec[:], in_=den[:])
                    agg = wk.tile([128, H], f32, tag="agg")
                    nc.vector.tensor_tensor(
                        out=agg[:], in0=ps_agg[:, 0:H], in1=rec[:], op=OP.mult
                    )
                    # ---- MLP ----
                    sl = slice(b * H, (b + 1) * H)
                    u = wk.tile([128, H], f32, tag="u")
                    nc.vector.tensor_tensor(
                        out=u[:], in0=agg[:], in1=r_sb[:, sl], op=OP.add
                    )
                    ps_t = pq.tile([H, 128], f32, tag="ptr")
                    nc.tensor.transpose(out=ps_t[:], in_=u[:], identity=ident[:])
                    uT = wk.tile([H, 128], f32, tag="uT")
                    nc.scalar.copy(out=uT[:], in_=ps_t[:])
                    ps_h1 = pq.tile([128, H2], f32, tag="ph1")
                    nc.tensor.matmul(
                        out=ps_h1[:], lhsT=uT[:], rhs=w1_sb[:], start=True, stop=True
                    )
                    l2 = slice(l * H2, (l + 1) * H2)
                    h1 = wk.tile([128, H2], f32, tag="h1")
                    nc.vector.tensor_tensor(
                        out=h1[:], in0=ps_h1[:], in1=b1r_sb[:, l2], op=OP.add
                    )
                    if l < 4:
                        # conv LayerNorm over 128 feats
                        s1 = wk.tile([128, 1], f32, tag="cs1")
                        nc.vector.reduce_sum(out=s1[:], in_=h1[:], axis=AX.X)
                        mu = wk.tile([128, 1], f32, tag="cmu")
                        nc.vector.tensor_scalar_mul(
                            out=mu[:], in0=s1[:], scalar1=1.0 / H2
                        )
                        hc = wk.tile([128, H2], f32, tag="chc")
                        nc.vector.tensor_scalar_sub(out=hc[:], in0=h1[:], scalar1=mu[:])
                        sq = wk.tile([128, H2], f32, tag="csq")
                        nc.scalar.square(out=sq[:], in_=hc[:])
                        s2 = wk.tile([128, 1], f32, tag="cs2")
                        nc.vector.reduce_sum(out=s2[:], in_=sq[:], axis=AX.X)
                        sd = wk.tile([128, 1], f32, tag="csd")
                        nc.scalar.activation(
                            out=sd[:],
                            in_=s2[:],
                            func=AF.Sqrt,
                            bias=lneps_sb[:],
                            scale=1.0 / H2,
                        )
                        rstd = wk.tile([128, 1], f32, tag="crstd")
                        nc.vector.reciprocal(out=rstd[:], in_=sd[:])
                        hn = wk.tile([128, H2], f32, tag="chn")
                        nc.vector.tensor_scalar_mul(
                            out=hn[:], in0=hc[:], scalar1=rstd[:]
                        )
                    else:
                        hn = h1
                    hg = wk.tile([128, H2], f32, tag="chg")
                    nc.vector.tensor_tensor(
                        out=hg[:], in0=hn[:], in1=gar_sb[:, l2], op=OP.mult
                    )
                    hb2 = wk.tile([128, H2], f32, tag="chb")
                    nc.vector.tensor_tensor(
                        out=hb2[:], in0=hg[:], in1=bar_sb[:, l2], op=OP.add
                    )
                    r1 = wk.tile([128, H2], f32, tag="r1")
                    nc.vector.tensor_scalar_max(out=r1[:], in0=hb2[:], scalar1=0.0)
                    ps_t2 = pq.tile([128, 128], f32, tag="ptr2")
                    nc.tensor.transpose(out=ps_t2[:], in_=r1[:], identity=ident[:])
                    r1T = wk.tile([128, 128], f32, tag="r1T")
                    nc.scalar.copy(out=r1T[:], in_=ps_t2[:])
                    ps_h2 = pq.tile([128, H], f32, tag="ph2")
                    nc.tensor.matmul(
                        out=ps_h2[:], lhsT=r1T[:], rhs=w2_sb[:], start=True, stop=True
                    )
                    lh = slice(l * H, (l + 1) * H)
                    if l == 0:
                        nc.vector.tensor_tensor(
                            out=h_nxt[:, sl], in0=ps_h2[:], in1=b2r_sb[:, lh], op=OP.add
                        )
                    elif l < 4:
                        co = wk.tile([128, H], f32, tag="co")
                        nc.vector.tensor_tensor(
                            out=co[:], in0=ps_h2[:], in1=b2r_sb[:, lh], op=OP.add
                        )
                        nc.vector.tensor_tensor(
                            out=h_nxt[:, sl], in0=co[:], in1=h_cur[:, sl], op=OP.add
                        )
                    else:
                        co = wk.tile([128, H], f32, tag="co")
                        nc.vector.tensor_tensor(
                            out=co[:], in0=ps_h2[:], in1=b2r_sb[:, lh], op=OP.add
                        )
                        nc.sync.dma_start(
                            out=d_out[b * 128 : (b + 1) * 128, :], in_=co[:]
                        )
                    if l < 4:
                        # r for conv l+1 from h_nxt, then its T chunk
                        gsl = slice(l * H, (l + 1) * H)
                        layer_norm_64(
                            r_sb[:, sl],
                            h_nxt[:, sl],
                            blkg_sb[:, gsl],
                            blkb_sb[:, gsl],
                            relu=True,
                        )
                        t_chunk(b, l + 1)
                        if b == HB - 1:
                            push_half1(d_tabs[(l + 1) % 2])
                if l < 4:
                    push_half2(d_tabs[(l + 1) % 2])
                    h_cur, h_nxt = h_nxt, h_cur

    in_maps = []
    for c in range(NC):
        in_maps.append(
            {
                "xT": xT[c],
                "idxs": np.ascontiguousarray(idxs_np[c]),
                "oh": np.ascontiguousarray(oh_np[c]),
                "w1all": w1all,
                "w2all": w2all,
                "b1rep": b1rep,
                "garr": garr,
                "barr": barr,
                "b2rep": b2rep,
                "blkg": blkg,
                "blkb": blkb,
                "tsc": tsc,
                "lneps": np.full((128, 1), LN_EPS, np.float32),
                "tbi": tbi,
                "encW": encW,
                "encb": encb,
            }
        )
    lower_extended_insts(nc)
    res = run_bass_kernel_spmd(nc, in_maps, list(range(NC)), trace=_trace)
    LAST_EXEC_NS = res.exec_time_ns
    out = np.empty((N, H), dtype=np.float32)
    for c in range(NC):
        oc = res.results[c]["out"]
        for b in range(NB):
            q0, q1 = int(qbounds[c, b]), int(qbounds[c, b + 1])
            if q1 > q0:
                out[c * NPC + q0 : c * NPC + q1] = oc[b * 128 : b * 128 + (q1 - q0)]
    return out.astype(np.float32)

